# revision 13
# baseline (speedup 1.0000x reference)
"""Distributed GNN encoder (9x SAGEConv + BN) on 8 Trainium2 NeuronCores.

Scheme (validated in numpy sim, rel_l2 ~4e-3 vs reference):
- Nodes sharded: core c owns padded rows [1280c, 1280(c+1)) of 10240 (10000 real).
- Feature-major compute on device; per conv pass either:
    'pre'  (expanding): AllGather h'' (bf16, width din), gather + one-hot
           scatter-matmul aggregation per 128-node block, then agg@Wl + h@Wr.
    'post' (contracting): Y = h''@Wl locally (f32), AllGather Y (bf16),
           aggregate Y, transpose agg into conv PSUM + h@Wr.
- Aggregation: dma_gather rows from the AllGathered DRAM buffer (bf16) +
  one-hot scatter matmuls (bf16, exact 0/1) into PSUM, exact f32 inv-degree.
- BatchNorm: relu with accumulated stats (f32) -> small AllReduce -> fused
  mult-add normalize. Dense matmuls all f32.
Outputs (mu, logvar) assembled host-side from per-core [1280,128] blocks.
"""
import sys
sys.path.insert(0, "/opt/trn_rl_repo")
import numpy as np
import ml_dtypes

NC = 8
BLK = 128
NBLK = 10
NB = BLK * NBLK          # 1280
NPAD = NC * NB           # 10240
N = 10000
FEAT = 64
EPS = 1e-5
NREAL7 = N - 7 * NB      # 1040: real rows of core 7 (stat-mask split point)

DIMS = [(64, 256), (256, 512), (512, 1024), (1024, 1024), (1024, 512),
        (512, 256), (256, 128), (128, 64), (128, 64)]

# (layers, din, dagg, dout, mode, has_bn)
PASSES = [
    ([0], 64, 64, 256, 'pre', True),
    ([1], 256, 256, 512, 'pre', True),
    ([2], 512, 512, 1024, 'pre', True),
    ([3], 1024, 1024, 1024, 'post', True),
    ([4], 1024, 512, 512, 'post', True),
    ([5], 512, 256, 256, 'post', True),
    ([6], 256, 128, 128, 'post', True),
    ([7, 8], 128, 128, 128, 'post', False),
]

# node-column chunks (free dim of conv matmuls) with their 128-node blocks;
# the last is split at the core-7 real/pad boundary for stats masking
SUPERS = [(0, 512, range(0, 4)), (512, 512, range(4, 8)), (1024, 256, range(8, 10))]


# ---------------------------------------------------------------- host prep
def _wrap_idxs(idx):
    idx = np.asarray(idx, np.int16)
    w = idx.reshape(-1, 16).T.copy()            # [16, n/16]
    return np.tile(w, (8, 1)).astype(np.int16)  # [128, n/16]


def _prep_graph(edge_index):
    src = np.asarray(edge_index[0], np.int64)
    dst = np.asarray(edge_index[1], np.int64)
    cnt = np.bincount(dst, minlength=N).astype(np.float32)
    inv_full = (1.0 / np.maximum(cnt, 1.0)).astype(np.float32)

    core_of = dst // NB
    blk_of = (dst % NB) // BLK

    T_b = np.ones(NBLK, np.int64)
    for b in range(NBLK):
        for c in range(NC):
            e = int(((core_of == c) & (blk_of == b)).sum())
            T_b[b] = max(T_b[b], -(-e // BLK))
    TT = int(T_b.sum())
    off = np.concatenate([[0], np.cumsum(T_b)]).astype(np.int64)

    per_core = []
    for c in range(NC):
        idx_all = np.zeros(TT * BLK, np.int64)
        oh_all = np.zeros((BLK, TT * BLK), np.float32)
        for b in range(NBLK):
            sel = np.nonzero((core_of == c) & (blk_of == b))[0]
            sel = sel[np.argsort(dst[sel], kind='stable')]
            t0 = int(off[b])
            j = np.arange(len(sel))
            t = t0 + j // BLK
            r = j % BLK
            idx_all[t * BLK + r] = src[sel]
            np.add.at(oh_all, (r, t * BLK + dst[sel] - (c * NB + b * BLK)), 1.0)
        ic = np.zeros((BLK, NBLK), np.float32)
        for b in range(NBLK):
            gids = c * NB + b * BLK + np.arange(BLK)
            v = gids < N
            ic[v, b] = inv_full[gids[v]]
        per_core.append((_wrap_idxs(idx_all), oh_all.astype(ml_dtypes.bfloat16), ic))
    return per_core, T_b, off, TT


# ---------------------------------------------------------------- device build
def _build(TT, T_b, off, flags):
    import concourse.bacc as bacc
    import concourse.tile as tile
    import concourse.mybir as mybir
    dt = mybir.dt
    ALU = mybir.AluOpType
    ACT = mybir.ActivationFunctionType
    has_b, has_g, has_beta = flags

    nc = bacc.Bacc("TRN2", target_bir_lowering=False, debug=False,
                   num_devices=NC)

    # ---- external inputs ----
    x_full = nc.dram_tensor("x_full", [NPAD, FEAT], dt.float32, kind="ExternalInput")
    xT = nc.dram_tensor("xT", [FEAT, NB], dt.float32, kind="ExternalInput")
    idx_d = nc.dram_tensor("idx", [128, TT * 8], dt.int16, kind="ExternalInput")
    oneh_d = nc.dram_tensor("oneh", [128, TT * 128], dt.bfloat16, kind="ExternalInput")
    invc_d = nc.dram_tensor("invc", [128, NBLK], dt.float32, kind="ExternalInput")
    mask_d = nc.dram_tensor("maskc", [128, 1], dt.float32, kind="ExternalInput")
    ident_d = nc.dram_tensor("ident", [128, 128], dt.float32, kind="ExternalInput")
    Wl_d, Wr_d, aux_d = {}, {}, {}
    for p, (layers, din, dagg, dout, mode, has_bn) in enumerate(PASSES):
        Wl_d[p] = nc.dram_tensor(f"Wlp{p}", [din, dout], dt.float32, kind="ExternalInput")
        Wr_d[p] = nc.dram_tensor(f"Wrp{p}", [din, dout], dt.float32, kind="ExternalInput")
        jd = dout // 128
        if has_b:
            aux_d[f"b{p}"] = nc.dram_tensor(f"bcol{p}", [128, jd], dt.float32,
                                            kind="ExternalInput")
        if has_bn and has_g:
            aux_d[f"g{p}"] = nc.dram_tensor(f"gcol{p}", [128, jd], dt.float32,
                                            kind="ExternalInput")
        if has_bn and has_beta:
            aux_d[f"beta{p}"] = nc.dram_tensor(f"betacol{p}", [128, jd], dt.float32,
                                               kind="ExternalInput")
    out_d = nc.dram_tensor("out", [NB, 128], dt.float32, kind="ExternalOutput")

    # ---- dram internals (exchange + stat bounces) ----
    ag_in, ag_out, ar_in, ar_out = {}, {}, {}, {}
    for p, (layers, din, dagg, dout, mode, has_bn) in enumerate(PASSES):
        if p > 0:
            ag_in[p] = nc.dram_tensor(f"agin{p}", [NB, dagg], dt.bfloat16)
            ag_out[p] = nc.dram_tensor(f"agout{p}", [NPAD, dagg], dt.bfloat16,
                                       addr_space="Shared")
        if has_bn:
            jd = dout // 128
            ar_in[p] = nc.dram_tensor(f"arin{p}", [128, 2 * jd], dt.float32)
            ar_out[p] = nc.dram_tensor(f"arout{p}", [NC, 128, 2 * jd], dt.float32,
                                       addr_space="Shared")

    RG = [list(range(NC))]

    with tile.TileContext(nc) as tc:
        with (
            tc.tile_pool(name="const", bufs=1) as cpool,
            tc.tile_pool(name="h", bufs=2) as hpool,
            tc.tile_pool(name="g", bufs=3) as gpool,
            tc.tile_pool(name="w", bufs=2) as wpool,
            tc.tile_pool(name="aggsb", bufs=6) as apool,
            tc.tile_pool(name="aggT", bufs=2) as atpool,
            tc.tile_pool(name="yj", bufs=2) as ypool,
            tc.tile_pool(name="small", bufs=4) as spool,
            tc.tile_pool(name="stats", bufs=2) as stpool,
            tc.tile_pool(name="sq", bufs=2) as sqpool,
            tc.tile_pool(name="aps", bufs=4, space="PSUM") as aps,
            tc.tile_pool(name="cps", bufs=2, space="PSUM") as cps,
            tc.tile_pool(name="tps", bufs=2, space="PSUM") as tps,
        ):
            # constants
            idx_sb = cpool.tile([128, TT * 8], dt.int16)
            nc.sync.dma_start(idx_sb[:], idx_d[:])
            oneh_sb = cpool.tile([128, TT * 128], dt.bfloat16)
            nc.sync.dma_start(oneh_sb[:], oneh_d[:])
            invc_sb = cpool.tile([128, NBLK], dt.float32)
            nc.sync.dma_start(invc_sb[:], invc_d[:])
            mask_sb = cpool.tile([128, 1], dt.float32)
            nc.sync.dma_start(mask_sb[:], mask_d[:])
            ident_sb = cpool.tile([128, 128], dt.float32)
            nc.sync.dma_start(ident_sb[:], ident_d[:])
            aux_sb = {}
            for k, d in aux_d.items():
                t = cpool.tile(list(d.shape), dt.float32)
                nc.sync.dma_start(t[:], d[:])
                aux_sb[k] = t

            # initial h: x^T block (feature-major, 64 real partitions)
            hT = hpool.tile([128, 1, NB], dt.float32, tag="h")
            nc.sync.dma_start(hT[:FEAT, 0, :], xT[:])

            def stream_w(dram, din, j):
                """Load W[:, j*128:(j+1)*128] as a [128, KI, 128] f32 tile."""
                kp = min(din, 128)
                ki = max(din // 128, 1)
                wt = wpool.tile([128, ki, 128], dt.float32, tag="w")
                v = dram.rearrange("(kc p) d -> p kc d", p=kp)
                nc.sync.dma_start(wt[:kp, :, :], v[:, :, j * 128:(j + 1) * 128])
                return wt

            def tr128(dst_ap, src_ap, kw=128):
                """dst = src^T via PE transpose + copy (f32)."""
                trp = tps.tile([128, 128], dt.float32, tag="tps")
                nc.tensor.transpose(trp[:kw, :], src_ap, ident_sb[:])
                nc.vector.tensor_copy(dst_ap, trp[:kw, :])

            import os
            npass = int(os.environ.get("GNN_NPASS", len(PASSES)))
            for p, (layers, din, dagg, dout, mode, has_bn) in enumerate(
                    PASSES[:npass]):
                kp = min(din, 128)
                KI = max(din // 128, 1)
                JD = dout // 128

                # ---------- exchange payload ----------
                if mode == 'post':
                    # Y^T per j-chunk -> transpose -> bf16 -> ag_in
                    for j in range(JD):
                        wl = stream_w(Wl_d[p], din, j)
                        yj = ypool.tile([128, NB], dt.float32, tag="yj")
                        for (n0, ns, _bs) in SUPERS:
                            yp = cps.tile([128, ns], dt.float32, tag="cps")
                            for k in range(KI):
                                nc.tensor.matmul(
                                    yp[:], wl[:kp, k, :], hT[:kp, k, n0:n0 + ns],
                                    start=(k == 0), stop=(k == KI - 1))
                            nc.vector.tensor_copy(yj[:, n0:n0 + ns], yp[:])
                        for b in range(NBLK):
                            trp = tps.tile([128, 128], dt.float32, tag="tps")
                            nc.tensor.transpose(
                                trp[:], yj[:, b * 128:(b + 1) * 128], ident_sb[:])
                            tb = spool.tile([128, 128], dt.bfloat16, tag="tb")
                            nc.vector.tensor_copy(tb[:], trp[:])
                            nc.sync.dma_start(
                                ag_in[p][b * 128:(b + 1) * 128,
                                         j * 128:(j + 1) * 128], tb[:])
                elif p > 0:
                    # payload is h'' itself (din wide)
                    for j in range(KI):
                        for b in range(NBLK):
                            trp = tps.tile([128, 128], dt.float32, tag="tps")
                            nc.tensor.transpose(
                                trp[:], hT[:, j, b * 128:(b + 1) * 128], ident_sb[:])
                            tb = spool.tile([128, 128], dt.bfloat16, tag="tb")
                            nc.vector.tensor_copy(tb[:], trp[:])
                            nc.sync.dma_start(
                                ag_in[p][b * 128:(b + 1) * 128,
                                         j * 128:(j + 1) * 128], tb[:])
                if p > 0:
                    nc.gpsimd.collective_compute(
                        "AllGather", ALU.bypass,
                        ins=[ag_in[p][:, :]], outs=[ag_out[p][:, :]],
                        replica_groups=RG)
                    gsrc, gdt = ag_out[p], dt.bfloat16
                else:
                    gsrc, gdt = x_full, dt.float32

                # ---------- per-super aggregation + conv ----------
                raw = hpool.tile([128, JD, NB], dt.float32, tag="h")
                stats_acc = stpool.tile([128, JD, 8], dt.float32, tag="stats")
                halves = [(h * 512, min(512, dagg - h * 512))
                          for h in range((dagg + 511) // 512)]

                # flat gather groups across block boundaries (dma_gather caps
                # at 1024 idxs/instruction on HW); tiles of one group may
                # belong to different aggregation blocks
                GRP = 5 if dagg >= 1024 else 8

                def gather_tile(t):
                    """Return (tile_handle, local_index) for global tile t,
                    gathering its group on first touch."""
                    g0 = (t // GRP) * GRP
                    if g0 not in gat_cache:
                        gn = min(GRP, TT - g0)
                        gt = gpool.tile([128, gn, dagg], gdt, tag="g")
                        if os.environ.get("GNN_NO_GATHER"):
                            nc.vector.memset(gt[:], 0.0)
                        else:
                            nc.gpsimd.dma_gather(
                                gt[:], gsrc[:, :],
                                idx_sb[:, g0 * 8:(g0 + gn) * 8],
                                gn * 128, gn * 128, dagg)
                        if gdt != dt.bfloat16:
                            gb = gpool.tile([128, gn, dagg], dt.bfloat16,
                                            tag="gb")
                            nc.vector.tensor_copy(gb[:], gt[:])
                            gt = gb
                        gat_cache[g0] = gt
                    return gat_cache[g0], t - g0

                gat_cache = {}
                for (n0, ns, bs) in SUPERS:
                    agg_of = {}
                    if mode == 'pre':
                        aggT = atpool.tile([128, KI, ns], dt.float32, tag="aggT")
                    for b in bs:
                        tb0, tb1 = int(off[b]), int(off[b + 1])
                        ntile = tb1 - tb0
                        agg_sb = apool.tile([128, dagg], dt.float32, tag="aggsb")
                        for (h0, hw) in halves:
                            ap_ = aps.tile([128, hw], dt.float32, tag="aps")
                            for ti, t in enumerate(range(tb0, tb1)):
                                gt, tl = gather_tile(t)
                                nc.tensor.matmul(
                                    ap_[:],
                                    oneh_sb[:, t * 128:(t + 1) * 128],
                                    gt[:, tl, h0:h0 + hw],
                                    start=(ti == 0),
                                    stop=(ti == ntile - 1))
                            nc.vector.tensor_scalar(
                                agg_sb[:, h0:h0 + hw], ap_[:],
                                invc_sb[:, b:b + 1], None, ALU.mult)
                        if mode == 'pre':
                            boff = (b - bs[0]) * 128
                            for k in range(KI):
                                kw = min(128, dagg - k * 128)
                                tr128(aggT[:kw, k, boff:boff + 128],
                                      agg_sb[:, k * 128:k * 128 + kw], kw)
                        else:
                            agg_of[b] = agg_sb

                    for j in range(JD):
                        wr = stream_w(Wr_d[p], din, j)
                        if mode == 'pre':
                            wl = stream_w(Wl_d[p], dagg, j)
                        cp = cps.tile([128, ns], dt.float32, tag="cps")
                        for k in range(KI):
                            nc.tensor.matmul(
                                cp[:], wr[:kp, k, :], hT[:kp, k, n0:n0 + ns],
                                start=(k == 0), stop=False)
                        if mode == 'pre':
                            kia = max(dagg // 128, 1)
                            kpa = min(dagg, 128)
                            for k in range(kia):
                                nc.tensor.matmul(
                                    cp[:], wl[:kpa, k, :], aggT[:kpa, k, :ns],
                                    start=False, stop=(k == kia - 1))
                        else:
                            for bi, b in enumerate(bs):
                                nc.tensor.matmul(
                                    cp[:, bi * 128:(bi + 1) * 128],
                                    agg_of[b][:, j * 128:(j + 1) * 128],
                                    ident_sb[:], is_transpose=True,
                                    start=False, stop=(bi == len(bs) - 1))
                        # epilogue: bias + relu + stats, or plain store
                        bias_ap = aux_sb[f"b{p}"][:, j:j + 1] if has_b else 0.0
                        if has_bn:
                            if n0 < 1024:
                                sl = [(0, ns, n0 // 512)]
                            else:
                                sw0 = NREAL7 - 1024
                                sl = [(0, sw0, 2), (sw0, ns - sw0, 3)]
                            for (s0, sw, slot) in sl:
                                nc.scalar.activation(
                                    raw[:, j, n0 + s0:n0 + s0 + sw],
                                    cp[:, s0:s0 + sw], ACT.Relu,
                                    bias=bias_ap,
                                    accum_out=stats_acc[:, j, slot:slot + 1])
                                sq = sqpool.tile([128, 512], dt.float32, tag="sq")
                                nc.scalar.activation(
                                    sq[:, :sw],
                                    raw[:, j, n0 + s0:n0 + s0 + sw], ACT.Square,
                                    accum_out=stats_acc[:, j, 4 + slot:5 + slot])
                        else:
                            if has_b:
                                nc.vector.tensor_scalar(
                                    raw[:, j, n0:n0 + ns], cp[:], bias_ap, None,
                                    ALU.add)
                            else:
                                nc.vector.tensor_copy(raw[:, j, n0:n0 + ns], cp[:])

                # ---------- BN: stats allreduce + normalize ----------
                if has_bn:
                    stats_sb = stpool.tile([128, 2 * JD], dt.float32, tag="statsum")
                    for j in range(JD):
                        for half, base in ((0, 0), (JD, 4)):
                            t0 = spool.tile([128, 1], dt.float32, tag="sc")
                            nc.vector.tensor_scalar(
                                t0[:], stats_acc[:, j, base + 3:base + 4],
                                mask_sb[:, 0:1], None, ALU.mult)
                            nc.vector.tensor_tensor(
                                t0[:], t0[:], stats_acc[:, j, base:base + 1],
                                ALU.add)
                            nc.vector.tensor_tensor(
                                t0[:], t0[:], stats_acc[:, j, base + 1:base + 2],
                                ALU.add)
                            nc.vector.tensor_tensor(
                                stats_sb[:, half + j:half + j + 1], t0[:],
                                stats_acc[:, j, base + 2:base + 3], ALU.add)
                    arr = stpool.tile([128, 2 * JD], dt.float32, tag="statsum")
                    if os.environ.get("GNN_NO_AR"):
                        nc.vector.tensor_scalar(arr[:], stats_sb[:], float(NC),
                                                None, ALU.mult)
                    else:
                        # AllGather partials + local sum (cheaper than AllReduce)
                        nc.sync.dma_start(ar_in[p][:, :], stats_sb[:])
                        nc.gpsimd.collective_compute(
                            "AllGather", ALU.bypass,
                            ins=[ar_in[p][:, :]], outs=[ar_out[p][:, :, :]],
                            replica_groups=RG)
                        parts = stpool.tile([128, NC, 2 * JD], dt.float32,
                                            tag="statparts")
                        nc.sync.dma_start(
                            parts[:], ar_out[p].rearrange("r p s -> p r s")[:])
                        nc.vector.tensor_tensor(arr[:], parts[:, 0, :],
                                                parts[:, 1, :], ALU.add)
                        for r in range(2, NC):
                            nc.vector.tensor_tensor(arr[:], arr[:],
                                                    parts[:, r, :], ALU.add)
                    mu = spool.tile([128, JD], dt.float32, tag="mu")
                    nc.vector.tensor_scalar(mu[:], arr[:, 0:JD], 1.0 / N, None,
                                            ALU.mult)
                    va = spool.tile([128, JD], dt.float32, tag="va")
                    nc.vector.tensor_scalar(va[:], arr[:, JD:2 * JD], 1.0 / N,
                                            None, ALU.mult)
                    mu2 = spool.tile([128, JD], dt.float32, tag="mu2")
                    nc.scalar.square(mu2[:], mu[:])
                    nc.vector.tensor_tensor(va[:], va[:], mu2[:], ALU.subtract)
                    nc.vector.tensor_scalar(va[:], va[:], EPS, None, ALU.add)
                    nc.scalar.sqrt(va[:], va[:])
                    aa = spool.tile([128, JD], dt.float32, tag="aa")
                    nc.vector.reciprocal(aa[:], va[:])
                    if has_g:
                        nc.vector.tensor_tensor(aa[:], aa[:], aux_sb[f"g{p}"][:],
                                                ALU.mult)
                    cc = spool.tile([128, JD], dt.float32, tag="cc")
                    nc.vector.tensor_tensor(cc[:], mu[:], aa[:], ALU.mult)
                    nc.vector.tensor_scalar(cc[:], cc[:], -1.0, None, ALU.mult)
                    if has_beta:
                        nc.vector.tensor_tensor(cc[:], cc[:],
                                                aux_sb[f"beta{p}"][:], ALU.add)
                    for j in range(JD):
                        nc.vector.tensor_scalar(
                            raw[:, j, :], raw[:, j, :],
                            aa[:, j:j + 1], cc[:, j:j + 1], ALU.mult, ALU.add)
                hT = raw

            # ---------- final output: node-major [1280, 128] ----------
            for b in range(NBLK):
                trp = tps.tile([128, 128], dt.float32, tag="tps")
                nc.tensor.transpose(trp[:], hT[:, 0, b * 128:(b + 1) * 128],
                                    ident_sb[:])
                ot = spool.tile([128, 128], dt.float32, tag="ot")
                nc.vector.tensor_copy(ot[:], trp[:])
                nc.sync.dma_start(out_d[b * 128:(b + 1) * 128, :], ot[:])

    nc.compile()
    return nc


# ---------------------------------------------------------------- entry point
_CACHE = {}


def prep_all(x, edge_index, params):
    """Returns (build_args, in_maps) for the SPMD run."""
    x = np.asarray(x, np.float32)
    per_core, T_b, off, TT = _prep_graph(edge_index)

    P = {k: np.asarray(v, np.float32) for k, v in params.items()}
    has_b = any(np.any(P[f"b{li}"] != 0) for li in range(9))
    has_g = any(np.any(P[f"g{bi}"] != 1) for bi in range(7))
    has_beta = any(np.any(P[f"beta{bi}"] != 0) for bi in range(7))

    x_pad = np.zeros((NPAD, FEAT), np.float32)
    x_pad[:N] = x
    ident = np.eye(128, dtype=np.float32)

    def col_wrap(v):
        d = v.shape[0]
        return np.ascontiguousarray(v.reshape(d // 128, 128).T)

    common = {"x_full": x_pad, "ident": ident}
    for p, (layers, din, dagg, dout, mode, has_bn) in enumerate(PASSES):
        common[f"Wlp{p}"] = np.ascontiguousarray(
            np.concatenate([P[f"Wl{li}"] for li in layers], 1))
        common[f"Wrp{p}"] = np.ascontiguousarray(
            np.concatenate([P[f"Wr{li}"] for li in layers], 1))
        if has_b:
            common[f"bcol{p}"] = col_wrap(np.concatenate(
                [P[f"b{li}"] for li in layers]))
        if has_bn:
            li = layers[0]
            if has_g:
                common[f"gcol{p}"] = col_wrap(P[f"g{li}"])
            if has_beta:
                common[f"betacol{p}"] = col_wrap(P[f"beta{li}"])

    in_maps = []
    for c in range(NC):
        idx_w, oneh, invc = per_core[c]
        m = dict(common)
        m["xT"] = np.ascontiguousarray(x_pad[c * NB:(c + 1) * NB].T)
        m["idx"] = idx_w
        m["oneh"] = oneh
        m["invc"] = invc
        m["maskc"] = np.full((128, 1), 1.0 if c < NC - 1 else 0.0, np.float32)
        in_maps.append(m)

    return (TT, T_b, off, (has_b, has_g, has_beta)), in_maps


def kernel(x, edge_index, params):
    from concourse.bass_utils import run_bass_kernel_spmd

    (TT, T_b, off, flags), in_maps = prep_all(x, edge_index, params)
    key = (TT, tuple(int(t) for t in T_b), flags)
    if key not in _CACHE:
        _CACHE[key] = _build(TT, T_b, off, flags)
    nc = _CACHE[key]

    res = run_bass_kernel_spmd(nc, in_maps, list(range(NC)), trace=False)
    out = np.concatenate([res.results[c]["out"] for c in range(NC)], 0)[:N]
    return (np.ascontiguousarray(out[:, :64]),
            np.ascontiguousarray(out[:, 64:128]))


# revision 18
# speedup vs baseline: 1.0062x; 1.0062x over previous
"""Distributed GNN encoder (9x SAGEConv + BN) on 8 Trainium2 NeuronCores.

Scheme (validated in numpy sim, rel_l2 ~4e-3 vs reference):
- Nodes sharded: core c owns padded rows [1280c, 1280(c+1)) of 10240 (10000 real).
- Feature-major compute on device; per conv pass either:
    'pre'  (expanding): AllGather h'' (bf16, width din), gather + one-hot
           scatter-matmul aggregation per 128-node block, then agg@Wl + h@Wr.
    'post' (contracting): Y = h''@Wl locally (f32), AllGather Y (bf16),
           aggregate Y, transpose agg into conv PSUM + h@Wr.
- Aggregation: dma_gather rows from the AllGathered DRAM buffer (bf16) +
  one-hot scatter matmuls (bf16, exact 0/1) into PSUM, exact f32 inv-degree.
- BatchNorm: relu with accumulated stats (f32) -> small AllReduce -> fused
  mult-add normalize. Dense matmuls all f32.
Outputs (mu, logvar) assembled host-side from per-core [1280,128] blocks.
"""
import sys
sys.path.insert(0, "/opt/trn_rl_repo")
import numpy as np
import ml_dtypes

NC = 8
BLK = 128
NBLK = 10
NB = BLK * NBLK          # 1280
NPAD = NC * NB           # 10240
N = 10000
FEAT = 64
EPS = 1e-5
NREAL7 = N - 7 * NB      # 1040: real rows of core 7 (stat-mask split point)

DIMS = [(64, 256), (256, 512), (512, 1024), (1024, 1024), (1024, 512),
        (512, 256), (256, 128), (128, 64), (128, 64)]

# (layers, din, dagg, dout, mode, has_bn)
PASSES = [
    ([0], 64, 64, 256, 'pre', True),
    ([1], 256, 256, 512, 'pre', True),
    ([2], 512, 512, 1024, 'pre', True),
    ([3], 1024, 1024, 1024, 'post', True),
    ([4], 1024, 512, 512, 'post', True),
    ([5], 512, 256, 256, 'post', True),
    ([6], 256, 128, 128, 'post', True),
    ([7, 8], 128, 128, 128, 'post', False),
]

# node-column chunks (free dim of conv matmuls) with their 128-node blocks;
# the last is split at the core-7 real/pad boundary for stats masking
SUPERS = [(0, 512, range(0, 4)), (512, 512, range(4, 8)), (1024, 256, range(8, 10))]


# ---------------------------------------------------------------- host prep
def _wrap_idxs(idx):
    idx = np.asarray(idx, np.int16)
    w = idx.reshape(-1, 16).T.copy()            # [16, n/16]
    return np.tile(w, (8, 1)).astype(np.int16)  # [128, n/16]


def _prep_graph(edge_index):
    src = np.asarray(edge_index[0], np.int64)
    dst = np.asarray(edge_index[1], np.int64)
    cnt = np.bincount(dst, minlength=N).astype(np.float32)
    inv_full = (1.0 / np.maximum(cnt, 1.0)).astype(np.float32)

    core_of = dst // NB
    blk_of = (dst % NB) // BLK

    T_b = np.ones(NBLK, np.int64)
    for b in range(NBLK):
        for c in range(NC):
            e = int(((core_of == c) & (blk_of == b)).sum())
            T_b[b] = max(T_b[b], -(-e // BLK))
    TT = int(T_b.sum())
    off = np.concatenate([[0], np.cumsum(T_b)]).astype(np.int64)

    per_core = []
    for c in range(NC):
        idx_all = np.zeros(TT * BLK, np.int64)
        oh_all = np.zeros((BLK, TT * BLK), np.float32)
        for b in range(NBLK):
            sel = np.nonzero((core_of == c) & (blk_of == b))[0]
            sel = sel[np.argsort(dst[sel], kind='stable')]
            t0 = int(off[b])
            j = np.arange(len(sel))
            t = t0 + j // BLK
            r = j % BLK
            idx_all[t * BLK + r] = src[sel]
            np.add.at(oh_all, (r, t * BLK + dst[sel] - (c * NB + b * BLK)), 1.0)
        ic = np.zeros((BLK, NBLK), np.float32)
        for b in range(NBLK):
            gids = c * NB + b * BLK + np.arange(BLK)
            v = gids < N
            ic[v, b] = inv_full[gids[v]]
        per_core.append((_wrap_idxs(idx_all), oh_all.astype(ml_dtypes.bfloat16), ic))
    return per_core, T_b, off, TT


# ---------------------------------------------------------------- device build
def _build(TT, T_b, off, flags):
    import concourse.bacc as bacc
    import concourse.tile as tile
    import concourse.mybir as mybir
    dt = mybir.dt
    ALU = mybir.AluOpType
    ACT = mybir.ActivationFunctionType
    has_b, has_g, has_beta = flags

    nc = bacc.Bacc("TRN2", target_bir_lowering=False, debug=False,
                   num_devices=NC)

    # ---- external inputs ----
    x_full = nc.dram_tensor("x_full", [NPAD, FEAT], dt.float32, kind="ExternalInput")
    xT = nc.dram_tensor("xT", [FEAT, NB], dt.float32, kind="ExternalInput")
    idx_d = nc.dram_tensor("idx", [128, TT * 8], dt.int16, kind="ExternalInput")
    oneh_d = nc.dram_tensor("oneh", [128, TT * 128], dt.bfloat16, kind="ExternalInput")
    invc_d = nc.dram_tensor("invc", [128, NBLK], dt.float32, kind="ExternalInput")
    mask_d = nc.dram_tensor("maskc", [128, 1], dt.float32, kind="ExternalInput")
    ident_d = nc.dram_tensor("ident", [128, 128], dt.float32, kind="ExternalInput")
    Wl_d, Wr_d, aux_d = {}, {}, {}
    for p, (layers, din, dagg, dout, mode, has_bn) in enumerate(PASSES):
        Wl_d[p] = nc.dram_tensor(f"Wlp{p}", [din, dout], dt.float32, kind="ExternalInput")
        Wr_d[p] = nc.dram_tensor(f"Wrp{p}", [din, dout], dt.float32, kind="ExternalInput")
        jd = dout // 128
        if has_b:
            aux_d[f"b{p}"] = nc.dram_tensor(f"bcol{p}", [128, jd], dt.float32,
                                            kind="ExternalInput")
        if has_bn and has_g:
            aux_d[f"g{p}"] = nc.dram_tensor(f"gcol{p}", [128, jd], dt.float32,
                                            kind="ExternalInput")
        if has_bn and has_beta:
            aux_d[f"beta{p}"] = nc.dram_tensor(f"betacol{p}", [128, jd], dt.float32,
                                               kind="ExternalInput")
    out_d = nc.dram_tensor("out", [NB, 128], dt.float32, kind="ExternalOutput")

    # ---- dram internals (exchange + stat bounces) ----
    ag_in, ag_out, ar_in, ar_out = {}, {}, {}, {}
    for p, (layers, din, dagg, dout, mode, has_bn) in enumerate(PASSES):
        if p > 0:
            ag_in[p] = nc.dram_tensor(f"agin{p}", [NB, dagg], dt.bfloat16)
            ag_out[p] = nc.dram_tensor(f"agout{p}", [NPAD, dagg], dt.bfloat16,
                                       addr_space="Shared")
        if has_bn:
            jd = dout // 128
            ar_in[p] = nc.dram_tensor(f"arin{p}", [128, 2 * jd], dt.float32)
            ar_out[p] = nc.dram_tensor(f"arout{p}", [NC, 128, 2 * jd], dt.float32,
                                       addr_space="Shared")

    RG = [list(range(NC))]

    with tile.TileContext(nc) as tc:
        with (
            tc.tile_pool(name="const", bufs=1) as cpool,
            tc.tile_pool(name="h", bufs=2) as hpool,
            tc.tile_pool(name="g", bufs=3) as gpool,
            tc.tile_pool(name="w", bufs=2) as wpool,
            tc.tile_pool(name="aggsb", bufs=4) as apool,
            tc.tile_pool(name="aggT", bufs=1) as atpool,
            tc.tile_pool(name="yj", bufs=2) as ypool,
            tc.tile_pool(name="small", bufs=4) as spool,
            tc.tile_pool(name="stats", bufs=2) as stpool,
            tc.tile_pool(name="sq", bufs=2) as sqpool,
            tc.tile_pool(name="nm", bufs=NBLK + 2) as npool,
            tc.tile_pool(name="aps", bufs=4, space="PSUM") as aps,
            tc.tile_pool(name="cps", bufs=2, space="PSUM") as cps,
            tc.tile_pool(name="tps", bufs=2, space="PSUM") as tps,
        ):
            # constants
            idx_sb = cpool.tile([128, TT * 8], dt.int16)
            nc.sync.dma_start(idx_sb[:], idx_d[:])
            oneh_sb = cpool.tile([128, TT * 128], dt.bfloat16)
            nc.sync.dma_start(oneh_sb[:], oneh_d[:])
            invc_sb = cpool.tile([128, NBLK], dt.float32)
            nc.sync.dma_start(invc_sb[:], invc_d[:])
            mask_sb = cpool.tile([128, 1], dt.float32)
            nc.sync.dma_start(mask_sb[:], mask_d[:])
            ident_sb = cpool.tile([128, 128], dt.float32)
            nc.sync.dma_start(ident_sb[:], ident_d[:])
            aux_sb = {}
            for k, d in aux_d.items():
                t = cpool.tile(list(d.shape), dt.float32)
                nc.sync.dma_start(t[:], d[:])
                aux_sb[k] = t

            # initial h: x^T block (feature-major, 64 real partitions)
            hT = hpool.tile([128, 1, NB], dt.float32, tag="h")
            nc.sync.dma_start(hT[:FEAT, 0, :], xT[:])

            def stream_w(dram, din, j):
                """Load W[:, j*128:(j+1)*128] as a [128, KI, 128] f32 tile."""
                kp = min(din, 128)
                ki = max(din // 128, 1)
                wt = wpool.tile([128, ki, 128], dt.float32, tag="w")
                v = dram.rearrange("(kc p) d -> p kc d", p=kp)
                nc.sync.dma_start(wt[:kp, :, :], v[:, :, j * 128:(j + 1) * 128])
                return wt

            def tr128(dst_ap, src_ap, kw=128):
                """dst = src^T via PE transpose + copy (f32)."""
                trp = tps.tile([128, 128], dt.float32, tag="tps")
                nc.tensor.transpose(trp[:kw, :], src_ap, ident_sb[:])
                nc.vector.tensor_copy(dst_ap, trp[:kw, :])

            import os
            npass = int(os.environ.get("GNN_NPASS", len(PASSES)))
            for p, (layers, din, dagg, dout, mode, has_bn) in enumerate(
                    PASSES[:npass]):
                kp = min(din, 128)
                KI = max(din // 128, 1)
                JD = dout // 128

                # ---------- exchange payload ----------
                # per-block node-major staging tiles: ONE DMA per block
                # instead of one per (j, block) (SP-sequencer issue cost
                # dominated the profile)
                if p > 0:
                    hwn = min(dagg, 512)
                    nm = {}

                    def nm_write(b, j, trp):
                        """Stage transpose chunk; flush the 512-col half to
                        ag_in when complete (one DMA per block-half)."""
                        h = (j * 128) // 512
                        if (b, h) not in nm:
                            nm[(b, h)] = npool.tile([128, hwn], dt.bfloat16,
                                                    tag="nm", name=f"nm{p}_{b}_{h}")
                        c0 = j * 128 - h * 512
                        nc.vector.tensor_copy(nm[(b, h)][:, c0:c0 + 128], trp)
                        if c0 + 128 == hwn or (j + 1) * 128 == dagg:
                            nc.sync.dma_start(
                                ag_in[p][b * 128:(b + 1) * 128,
                                         h * 512:h * 512 + c0 + 128],
                                nm.pop((b, h))[:, :c0 + 128])
                if mode == 'post':
                    # Y^T per j-chunk -> transpose -> bf16 -> staging
                    for j in range(JD):
                        wl = stream_w(Wl_d[p], din, j)
                        yj = ypool.tile([128, NB], dt.float32, tag="yj")
                        for (n0, ns, _bs) in SUPERS:
                            yp = cps.tile([128, ns], dt.float32, tag="cps")
                            for k in range(KI):
                                nc.tensor.matmul(
                                    yp[:], wl[:kp, k, :], hT[:kp, k, n0:n0 + ns],
                                    start=(k == 0), stop=(k == KI - 1))
                            nc.vector.tensor_copy(yj[:, n0:n0 + ns], yp[:])
                        for b in range(NBLK):
                            trp = tps.tile([128, 128], dt.float32, tag="tps")
                            nc.tensor.transpose(
                                trp[:], yj[:, b * 128:(b + 1) * 128], ident_sb[:])
                            nm_write(b, j, trp[:])
                elif p > 0:
                    # payload is h'' itself (din wide)
                    for j in range(KI):
                        for b in range(NBLK):
                            trp = tps.tile([128, 128], dt.float32, tag="tps")
                            nc.tensor.transpose(
                                trp[:], hT[:, j, b * 128:(b + 1) * 128], ident_sb[:])
                            nm_write(b, j, trp[:])
                if p > 0:
                    nc.gpsimd.collective_compute(
                        "AllGather", ALU.bypass,
                        ins=[ag_in[p][:, :]], outs=[ag_out[p][:, :]],
                        replica_groups=RG)
                    gsrc, gdt = ag_out[p], dt.bfloat16
                else:
                    gsrc, gdt = x_full, dt.float32

                # ---------- per-super aggregation + conv ----------
                raw = hpool.tile([128, JD, NB], dt.float32, tag="h")
                stats_acc = stpool.tile([128, JD, 8], dt.float32, tag="stats")
                halves = [(h * 512, min(512, dagg - h * 512))
                          for h in range((dagg + 511) // 512)]

                # flat gather groups across block boundaries (dma_gather caps
                # at 1024 idxs/instruction on HW); tiles of one group may
                # belong to different aggregation blocks
                GRP = 5 if dagg >= 1024 else 8

                def gather_tile(t):
                    """Return (tile_handle, local_index) for global tile t,
                    gathering its group on first touch."""
                    g0 = (t // GRP) * GRP
                    if g0 not in gat_cache:
                        gn = min(GRP, TT - g0)
                        gt = gpool.tile([128, gn, dagg], gdt, tag="g")
                        if os.environ.get("GNN_NO_GATHER"):
                            nc.vector.memset(gt[:], 0.0)
                        else:
                            nc.gpsimd.dma_gather(
                                gt[:], gsrc[:, :],
                                idx_sb[:, g0 * 8:(g0 + gn) * 8],
                                gn * 128, gn * 128, dagg)
                        if gdt != dt.bfloat16:
                            gb = gpool.tile([128, gn, dagg], dt.bfloat16,
                                            tag="gb")
                            nc.vector.tensor_copy(gb[:], gt[:])
                            gt = gb
                        gat_cache[g0] = gt
                    return gat_cache[g0], t - g0

                gat_cache = {}
                for (n0, ns, bs) in SUPERS:
                    agg_of = {}
                    if mode == 'pre':
                        aggT = atpool.tile([128, KI, ns], dt.float32, tag="aggT")
                    for b in bs:
                        tb0, tb1 = int(off[b]), int(off[b + 1])
                        ntile = tb1 - tb0
                        agg_sb = apool.tile([128, dagg], dt.float32, tag="aggsb")
                        for (h0, hw) in halves:
                            ap_ = aps.tile([128, hw], dt.float32, tag="aps")
                            for ti, t in enumerate(range(tb0, tb1)):
                                gt, tl = gather_tile(t)
                                nc.tensor.matmul(
                                    ap_[:],
                                    oneh_sb[:, t * 128:(t + 1) * 128],
                                    gt[:, tl, h0:h0 + hw],
                                    start=(ti == 0),
                                    stop=(ti == ntile - 1))
                            nc.vector.tensor_scalar(
                                agg_sb[:, h0:h0 + hw], ap_[:],
                                invc_sb[:, b:b + 1], None, ALU.mult)
                        if mode == 'pre':
                            boff = (b - bs[0]) * 128
                            for k in range(KI):
                                kw = min(128, dagg - k * 128)
                                tr128(aggT[:kw, k, boff:boff + 128],
                                      agg_sb[:, k * 128:k * 128 + kw], kw)
                        else:
                            agg_of[b] = agg_sb

                    for j in range(JD):
                        wr = stream_w(Wr_d[p], din, j)
                        if mode == 'pre':
                            wl = stream_w(Wl_d[p], dagg, j)
                        cp = cps.tile([128, ns], dt.float32, tag="cps")
                        for k in range(KI):
                            nc.tensor.matmul(
                                cp[:], wr[:kp, k, :], hT[:kp, k, n0:n0 + ns],
                                start=(k == 0), stop=False)
                        if mode == 'pre':
                            kia = max(dagg // 128, 1)
                            kpa = min(dagg, 128)
                            for k in range(kia):
                                nc.tensor.matmul(
                                    cp[:], wl[:kpa, k, :], aggT[:kpa, k, :ns],
                                    start=False, stop=(k == kia - 1))
                        else:
                            for bi, b in enumerate(bs):
                                nc.tensor.matmul(
                                    cp[:, bi * 128:(bi + 1) * 128],
                                    agg_of[b][:, j * 128:(j + 1) * 128],
                                    ident_sb[:], is_transpose=True,
                                    start=False, stop=(bi == len(bs) - 1))
                        # epilogue: bias + relu + stats, or plain store
                        bias_ap = aux_sb[f"b{p}"][:, j:j + 1] if has_b else 0.0
                        if has_bn:
                            if n0 < 1024:
                                sl = [(0, ns, n0 // 512)]
                            else:
                                sw0 = NREAL7 - 1024
                                sl = [(0, sw0, 2), (sw0, ns - sw0, 3)]
                            for (s0, sw, slot) in sl:
                                nc.scalar.activation(
                                    raw[:, j, n0 + s0:n0 + s0 + sw],
                                    cp[:, s0:s0 + sw], ACT.Relu,
                                    bias=bias_ap,
                                    accum_out=stats_acc[:, j, slot:slot + 1])
                                sq = sqpool.tile([128, 512], dt.float32, tag="sq")
                                nc.scalar.activation(
                                    sq[:, :sw],
                                    raw[:, j, n0 + s0:n0 + s0 + sw], ACT.Square,
                                    accum_out=stats_acc[:, j, 4 + slot:5 + slot])
                        else:
                            if has_b:
                                nc.vector.tensor_scalar(
                                    raw[:, j, n0:n0 + ns], cp[:], bias_ap, None,
                                    ALU.add)
                            else:
                                nc.vector.tensor_copy(raw[:, j, n0:n0 + ns], cp[:])

                # ---------- BN: stats allreduce + normalize ----------
                if has_bn:
                    stats_sb = stpool.tile([128, 2 * JD], dt.float32, tag="statsum")
                    for j in range(JD):
                        for half, base in ((0, 0), (JD, 4)):
                            t0 = spool.tile([128, 1], dt.float32, tag="sc")
                            nc.vector.tensor_scalar(
                                t0[:], stats_acc[:, j, base + 3:base + 4],
                                mask_sb[:, 0:1], None, ALU.mult)
                            nc.vector.tensor_tensor(
                                t0[:], t0[:], stats_acc[:, j, base:base + 1],
                                ALU.add)
                            nc.vector.tensor_tensor(
                                t0[:], t0[:], stats_acc[:, j, base + 1:base + 2],
                                ALU.add)
                            nc.vector.tensor_tensor(
                                stats_sb[:, half + j:half + j + 1], t0[:],
                                stats_acc[:, j, base + 2:base + 3], ALU.add)
                    arr = stpool.tile([128, 2 * JD], dt.float32, tag="statsum")
                    if os.environ.get("GNN_NO_AR"):
                        nc.vector.tensor_scalar(arr[:], stats_sb[:], float(NC),
                                                None, ALU.mult)
                    else:
                        # AllGather partials + local sum (cheaper than AllReduce)
                        nc.sync.dma_start(ar_in[p][:, :], stats_sb[:])
                        nc.gpsimd.collective_compute(
                            "AllGather", ALU.bypass,
                            ins=[ar_in[p][:, :]], outs=[ar_out[p][:, :, :]],
                            replica_groups=RG)
                        parts = stpool.tile([128, NC, 2 * JD], dt.float32,
                                            tag="statparts")
                        nc.sync.dma_start(
                            parts[:], ar_out[p].rearrange("r p s -> p r s")[:])
                        nc.vector.tensor_tensor(arr[:], parts[:, 0, :],
                                                parts[:, 1, :], ALU.add)
                        for r in range(2, NC):
                            nc.vector.tensor_tensor(arr[:], arr[:],
                                                    parts[:, r, :], ALU.add)
                    mu = spool.tile([128, JD], dt.float32, tag="mu")
                    nc.vector.tensor_scalar(mu[:], arr[:, 0:JD], 1.0 / N, None,
                                            ALU.mult)
                    va = spool.tile([128, JD], dt.float32, tag="va")
                    nc.vector.tensor_scalar(va[:], arr[:, JD:2 * JD], 1.0 / N,
                                            None, ALU.mult)
                    mu2 = spool.tile([128, JD], dt.float32, tag="mu2")
                    nc.scalar.square(mu2[:], mu[:])
                    nc.vector.tensor_tensor(va[:], va[:], mu2[:], ALU.subtract)
                    nc.vector.tensor_scalar(va[:], va[:], EPS, None, ALU.add)
                    nc.scalar.sqrt(va[:], va[:])
                    aa = spool.tile([128, JD], dt.float32, tag="aa")
                    nc.vector.reciprocal(aa[:], va[:])
                    if has_g:
                        nc.vector.tensor_tensor(aa[:], aa[:], aux_sb[f"g{p}"][:],
                                                ALU.mult)
                    cc = spool.tile([128, JD], dt.float32, tag="cc")
                    nc.vector.tensor_tensor(cc[:], mu[:], aa[:], ALU.mult)
                    nc.vector.tensor_scalar(cc[:], cc[:], -1.0, None, ALU.mult)
                    if has_beta:
                        nc.vector.tensor_tensor(cc[:], cc[:],
                                                aux_sb[f"beta{p}"][:], ALU.add)
                    for j in range(JD):
                        nc.vector.tensor_scalar(
                            raw[:, j, :], raw[:, j, :],
                            aa[:, j:j + 1], cc[:, j:j + 1], ALU.mult, ALU.add)
                hT = raw

            # ---------- final output: node-major [1280, 128] ----------
            for b in range(NBLK):
                trp = tps.tile([128, 128], dt.float32, tag="tps")
                nc.tensor.transpose(trp[:], hT[:, 0, b * 128:(b + 1) * 128],
                                    ident_sb[:])
                ot = spool.tile([128, 128], dt.float32, tag="ot")
                nc.vector.tensor_copy(ot[:], trp[:])
                nc.sync.dma_start(out_d[b * 128:(b + 1) * 128, :], ot[:])

    nc.compile()
    return nc


# ---------------------------------------------------------------- entry point
_CACHE = {}


def prep_all(x, edge_index, params):
    """Returns (build_args, in_maps) for the SPMD run."""
    x = np.asarray(x, np.float32)
    per_core, T_b, off, TT = _prep_graph(edge_index)

    P = {k: np.asarray(v, np.float32) for k, v in params.items()}
    has_b = any(np.any(P[f"b{li}"] != 0) for li in range(9))
    has_g = any(np.any(P[f"g{bi}"] != 1) for bi in range(7))
    has_beta = any(np.any(P[f"beta{bi}"] != 0) for bi in range(7))

    x_pad = np.zeros((NPAD, FEAT), np.float32)
    x_pad[:N] = x
    ident = np.eye(128, dtype=np.float32)

    def col_wrap(v):
        d = v.shape[0]
        return np.ascontiguousarray(v.reshape(d // 128, 128).T)

    common = {"x_full": x_pad, "ident": ident}
    for p, (layers, din, dagg, dout, mode, has_bn) in enumerate(PASSES):
        common[f"Wlp{p}"] = np.ascontiguousarray(
            np.concatenate([P[f"Wl{li}"] for li in layers], 1))
        common[f"Wrp{p}"] = np.ascontiguousarray(
            np.concatenate([P[f"Wr{li}"] for li in layers], 1))
        if has_b:
            common[f"bcol{p}"] = col_wrap(np.concatenate(
                [P[f"b{li}"] for li in layers]))
        if has_bn:
            li = layers[0]
            if has_g:
                common[f"gcol{p}"] = col_wrap(P[f"g{li}"])
            if has_beta:
                common[f"betacol{p}"] = col_wrap(P[f"beta{li}"])

    in_maps = []
    for c in range(NC):
        idx_w, oneh, invc = per_core[c]
        m = dict(common)
        m["xT"] = np.ascontiguousarray(x_pad[c * NB:(c + 1) * NB].T)
        m["idx"] = idx_w
        m["oneh"] = oneh
        m["invc"] = invc
        m["maskc"] = np.full((128, 1), 1.0 if c < NC - 1 else 0.0, np.float32)
        in_maps.append(m)

    return (TT, T_b, off, (has_b, has_g, has_beta)), in_maps


def kernel(x, edge_index, params):
    from concourse.bass_utils import run_bass_kernel_spmd

    (TT, T_b, off, flags), in_maps = prep_all(x, edge_index, params)
    key = (TT, tuple(int(t) for t in T_b), flags)
    if key not in _CACHE:
        _CACHE[key] = _build(TT, T_b, off, flags)
    nc = _CACHE[key]

    res = run_bass_kernel_spmd(nc, in_maps, list(range(NC)), trace=False)
    out = np.concatenate([res.results[c]["out"] for c in range(NC)], 0)[:N]
    return (np.ascontiguousarray(out[:, :64]),
            np.ascontiguousarray(out[:, 64:128]))


# revision 19
# speedup vs baseline: 1.0125x; 1.0063x over previous
"""Distributed GNN encoder (9x SAGEConv + BN) on 8 Trainium2 NeuronCores.

Scheme (validated in numpy sim, rel_l2 ~4e-3 vs reference):
- Nodes sharded: core c owns padded rows [1280c, 1280(c+1)) of 10240 (10000 real).
- Feature-major compute on device; per conv pass either:
    'pre'  (expanding): AllGather h'' (bf16, width din), gather + one-hot
           scatter-matmul aggregation per 128-node block, then agg@Wl + h@Wr.
    'post' (contracting): Y = h''@Wl locally (f32), AllGather Y (bf16),
           aggregate Y, transpose agg into conv PSUM + h@Wr.
- Aggregation: dma_gather rows from the AllGathered DRAM buffer (bf16) +
  one-hot scatter matmuls (bf16, exact 0/1) into PSUM, exact f32 inv-degree.
- BatchNorm: relu with accumulated stats (f32) -> small AllReduce -> fused
  mult-add normalize. Dense matmuls all f32.
Outputs (mu, logvar) assembled host-side from per-core [1280,128] blocks.
"""
import sys
sys.path.insert(0, "/opt/trn_rl_repo")
import numpy as np
import ml_dtypes

NC = 8
BLK = 128
NBLK = 10
NB = BLK * NBLK          # 1280
NPAD = NC * NB           # 10240
N = 10000
FEAT = 64
EPS = 1e-5
NREAL7 = N - 7 * NB      # 1040: real rows of core 7 (stat-mask split point)

DIMS = [(64, 256), (256, 512), (512, 1024), (1024, 1024), (1024, 512),
        (512, 256), (256, 128), (128, 64), (128, 64)]

# (layers, din, dagg, dout, mode, has_bn)
PASSES = [
    ([0], 64, 64, 256, 'pre', True),
    ([1], 256, 256, 512, 'pre', True),
    ([2], 512, 512, 1024, 'pre', True),
    ([3], 1024, 1024, 1024, 'post', True),
    ([4], 1024, 512, 512, 'post', True),
    ([5], 512, 256, 256, 'post', True),
    ([6], 256, 128, 128, 'post', True),
    ([7, 8], 128, 128, 128, 'post', False),
]

# node-column chunks (free dim of conv matmuls) with their 128-node blocks;
# the last is split at the core-7 real/pad boundary for stats masking
SUPERS = [(0, 512, range(0, 4)), (512, 512, range(4, 8)), (1024, 256, range(8, 10))]


# ---------------------------------------------------------------- host prep
def _wrap_idxs(idx):
    idx = np.asarray(idx, np.int16)
    w = idx.reshape(-1, 16).T.copy()            # [16, n/16]
    return np.tile(w, (8, 1)).astype(np.int16)  # [128, n/16]


def _prep_graph(edge_index):
    src = np.asarray(edge_index[0], np.int64)
    dst = np.asarray(edge_index[1], np.int64)
    cnt = np.bincount(dst, minlength=N).astype(np.float32)
    inv_full = (1.0 / np.maximum(cnt, 1.0)).astype(np.float32)

    core_of = dst // NB
    blk_of = (dst % NB) // BLK

    T_b = np.ones(NBLK, np.int64)
    for b in range(NBLK):
        for c in range(NC):
            e = int(((core_of == c) & (blk_of == b)).sum())
            T_b[b] = max(T_b[b], -(-e // BLK))
    TT = int(T_b.sum())
    off = np.concatenate([[0], np.cumsum(T_b)]).astype(np.int64)

    per_core = []
    for c in range(NC):
        idx_all = np.zeros(TT * BLK, np.int64)
        oh_all = np.zeros((BLK, TT * BLK), np.float32)
        for b in range(NBLK):
            sel = np.nonzero((core_of == c) & (blk_of == b))[0]
            sel = sel[np.argsort(dst[sel], kind='stable')]
            t0 = int(off[b])
            j = np.arange(len(sel))
            t = t0 + j // BLK
            r = j % BLK
            idx_all[t * BLK + r] = src[sel]
            np.add.at(oh_all, (r, t * BLK + dst[sel] - (c * NB + b * BLK)), 1.0)
        ic = np.zeros((BLK, NBLK), np.float32)
        for b in range(NBLK):
            gids = c * NB + b * BLK + np.arange(BLK)
            v = gids < N
            ic[v, b] = inv_full[gids[v]]
        per_core.append((_wrap_idxs(idx_all), oh_all.astype(ml_dtypes.bfloat16), ic))
    return per_core, T_b, off, TT


# ---------------------------------------------------------------- device build
def _build(TT, T_b, off, flags):
    import concourse.bacc as bacc
    import concourse.tile as tile
    import concourse.mybir as mybir
    dt = mybir.dt
    ALU = mybir.AluOpType
    ACT = mybir.ActivationFunctionType
    has_b, has_g, has_beta = flags

    nc = bacc.Bacc("TRN2", target_bir_lowering=False, debug=False,
                   num_devices=NC)

    # ---- external inputs ----
    x_full = nc.dram_tensor("x_full", [NPAD, FEAT], dt.float32, kind="ExternalInput")
    xT = nc.dram_tensor("xT", [FEAT, NB], dt.float32, kind="ExternalInput")
    idx_d = nc.dram_tensor("idx", [128, TT * 8], dt.int16, kind="ExternalInput")
    oneh_d = nc.dram_tensor("oneh", [128, TT * 128], dt.bfloat16, kind="ExternalInput")
    invc_d = nc.dram_tensor("invc", [128, NBLK], dt.float32, kind="ExternalInput")
    mask_d = nc.dram_tensor("maskc", [128, 1], dt.float32, kind="ExternalInput")
    ident_d = nc.dram_tensor("ident", [128, 128], dt.float32, kind="ExternalInput")
    Wl_d, Wr_d, aux_d = {}, {}, {}
    for p, (layers, din, dagg, dout, mode, has_bn) in enumerate(PASSES):
        Wl_d[p] = nc.dram_tensor(f"Wlp{p}", [din, dout], dt.float32, kind="ExternalInput")
        Wr_d[p] = nc.dram_tensor(f"Wrp{p}", [din, dout], dt.float32, kind="ExternalInput")
        jd = dout // 128
        if has_b:
            aux_d[f"b{p}"] = nc.dram_tensor(f"bcol{p}", [128, jd], dt.float32,
                                            kind="ExternalInput")
        if has_bn and has_g:
            aux_d[f"g{p}"] = nc.dram_tensor(f"gcol{p}", [128, jd], dt.float32,
                                            kind="ExternalInput")
        if has_bn and has_beta:
            aux_d[f"beta{p}"] = nc.dram_tensor(f"betacol{p}", [128, jd], dt.float32,
                                               kind="ExternalInput")
    out_d = nc.dram_tensor("out", [NB, 128], dt.float32, kind="ExternalOutput")

    # ---- dram internals (exchange + stat bounces) ----
    ag_in, ag_out, ar_in, ar_out = {}, {}, {}, {}
    for p, (layers, din, dagg, dout, mode, has_bn) in enumerate(PASSES):
        if p > 0:
            ag_in[p] = nc.dram_tensor(f"agin{p}", [NB, dagg], dt.bfloat16)
            ag_out[p] = nc.dram_tensor(f"agout{p}", [NPAD, dagg], dt.bfloat16,
                                       addr_space="Shared")
        if has_bn:
            jd = dout // 128
            ar_in[p] = nc.dram_tensor(f"arin{p}", [128, 2 * jd], dt.float32)
            ar_out[p] = nc.dram_tensor(f"arout{p}", [NC, 128, 2 * jd], dt.float32,
                                       addr_space="Shared")

    RG = [list(range(NC))]

    with tile.TileContext(nc) as tc:
        with (
            tc.tile_pool(name="const", bufs=1) as cpool,
            tc.tile_pool(name="h", bufs=2) as hpool,
            tc.tile_pool(name="g", bufs=2) as gpool,
            tc.tile_pool(name="w", bufs=2) as wpool,
            tc.tile_pool(name="aggsb", bufs=4) as apool,
            tc.tile_pool(name="aggT", bufs=1) as atpool,
            tc.tile_pool(name="yj", bufs=2) as ypool,
            tc.tile_pool(name="small", bufs=4) as spool,
            tc.tile_pool(name="stats", bufs=2) as stpool,
            tc.tile_pool(name="sq", bufs=2) as sqpool,
            tc.tile_pool(name="nm", bufs=NBLK + 2) as npool,
            tc.tile_pool(name="aps", bufs=3, space="PSUM") as aps,
            tc.tile_pool(name="cps", bufs=3, space="PSUM") as cps,
            tc.tile_pool(name="tps", bufs=2, space="PSUM") as tps,
        ):
            # constants
            idx_sb = cpool.tile([128, TT * 8], dt.int16)
            nc.sync.dma_start(idx_sb[:], idx_d[:])
            oneh_sb = cpool.tile([128, TT * 128], dt.bfloat16)
            nc.sync.dma_start(oneh_sb[:], oneh_d[:])
            invc_sb = cpool.tile([128, NBLK], dt.float32)
            nc.sync.dma_start(invc_sb[:], invc_d[:])
            mask_sb = cpool.tile([128, 1], dt.float32)
            nc.sync.dma_start(mask_sb[:], mask_d[:])
            ident_sb = cpool.tile([128, 128], dt.float32)
            nc.sync.dma_start(ident_sb[:], ident_d[:])
            aux_sb = {}
            for k, d in aux_d.items():
                t = cpool.tile(list(d.shape), dt.float32)
                nc.sync.dma_start(t[:], d[:])
                aux_sb[k] = t

            # initial h: x^T block (feature-major, 64 real partitions)
            hT = hpool.tile([128, 1, NB], dt.float32, tag="h")
            nc.sync.dma_start(hT[:FEAT, 0, :], xT[:])

            def stream_w(dram, din, j):
                """Load W[:, j*128:(j+1)*128] as a [128, KI, 128] f32 tile."""
                kp = min(din, 128)
                ki = max(din // 128, 1)
                wt = wpool.tile([128, ki, 128], dt.float32, tag="w")
                v = dram.rearrange("(kc p) d -> p kc d", p=kp)
                nc.sync.dma_start(wt[:kp, :, :], v[:, :, j * 128:(j + 1) * 128])
                return wt

            def tr128(dst_ap, src_ap, kw=128):
                """dst = src^T via PE transpose + copy (f32)."""
                trp = tps.tile([128, 128], dt.float32, tag="tps")
                nc.tensor.transpose(trp[:kw, :], src_ap, ident_sb[:])
                nc.vector.tensor_copy(dst_ap, trp[:kw, :])

            import os
            npass = int(os.environ.get("GNN_NPASS", len(PASSES)))
            for p, (layers, din, dagg, dout, mode, has_bn) in enumerate(
                    PASSES[:npass]):
                kp = min(din, 128)
                KI = max(din // 128, 1)
                JD = dout // 128

                # ---------- exchange payload ----------
                # per-block node-major staging tiles: ONE DMA per block
                # instead of one per (j, block) (SP-sequencer issue cost
                # dominated the profile)
                if p > 0:
                    hwn = min(dagg, 512)
                    nm = {}

                    def nm_write(b, j, trp):
                        """Stage transpose chunk; flush the 512-col half to
                        ag_in when complete (one DMA per block-half)."""
                        h = (j * 128) // 512
                        if (b, h) not in nm:
                            nm[(b, h)] = npool.tile([128, hwn], dt.bfloat16,
                                                    tag="nm", name=f"nm{p}_{b}_{h}")
                        c0 = j * 128 - h * 512
                        nc.vector.tensor_copy(nm[(b, h)][:, c0:c0 + 128], trp)
                        if c0 + 128 == hwn or (j + 1) * 128 == dagg:
                            nc.sync.dma_start(
                                ag_in[p][b * 128:(b + 1) * 128,
                                         h * 512:h * 512 + c0 + 128],
                                nm.pop((b, h))[:, :c0 + 128])
                if mode == 'post':
                    # Y^T per j-chunk -> transpose -> bf16 -> staging
                    for j in range(JD):
                        wl = stream_w(Wl_d[p], din, j)
                        yj = ypool.tile([128, NB], dt.float32, tag="yj")
                        for (n0, ns, _bs) in SUPERS:
                            yp = cps.tile([128, ns], dt.float32, tag="cps")
                            for k in range(KI):
                                nc.tensor.matmul(
                                    yp[:], wl[:kp, k, :], hT[:kp, k, n0:n0 + ns],
                                    start=(k == 0), stop=(k == KI - 1))
                            nc.vector.tensor_copy(yj[:, n0:n0 + ns], yp[:])
                        for b in range(NBLK):
                            trp = tps.tile([128, 128], dt.float32, tag="tps")
                            nc.tensor.transpose(
                                trp[:], yj[:, b * 128:(b + 1) * 128], ident_sb[:])
                            nm_write(b, j, trp[:])
                elif p > 0:
                    # payload is h'' itself (din wide)
                    for j in range(KI):
                        for b in range(NBLK):
                            trp = tps.tile([128, 128], dt.float32, tag="tps")
                            nc.tensor.transpose(
                                trp[:], hT[:, j, b * 128:(b + 1) * 128], ident_sb[:])
                            nm_write(b, j, trp[:])
                if p > 0:
                    nc.gpsimd.collective_compute(
                        "AllGather", ALU.bypass,
                        ins=[ag_in[p][:, :]], outs=[ag_out[p][:, :]],
                        replica_groups=RG)
                    gsrc, gdt = ag_out[p], dt.bfloat16
                else:
                    gsrc, gdt = x_full, dt.float32

                # ---------- per-super aggregation + conv ----------
                raw = hpool.tile([128, JD, NB], dt.float32, tag="h")
                stats_acc = stpool.tile([128, JD, 8], dt.float32, tag="stats")
                halves = [(h * 512, min(512, dagg - h * 512))
                          for h in range((dagg + 511) // 512)]

                # flat gather groups across block boundaries (dma_gather caps
                # at 1024 idxs/instruction on HW); tiles of one group may
                # belong to different aggregation blocks
                GRP = 8

                def gather_tile(t):
                    """Return (tile_handle, local_index) for global tile t,
                    gathering its group on first touch."""
                    g0 = (t // GRP) * GRP
                    if g0 not in gat_cache:
                        gn = min(GRP, TT - g0)
                        gt = gpool.tile([128, gn, dagg], gdt, tag="g")
                        if os.environ.get("GNN_NO_GATHER"):
                            nc.vector.memset(gt[:], 0.0)
                        else:
                            nc.gpsimd.dma_gather(
                                gt[:], gsrc[:, :],
                                idx_sb[:, g0 * 8:(g0 + gn) * 8],
                                gn * 128, gn * 128, dagg)
                        if gdt != dt.bfloat16:
                            gb = gpool.tile([128, gn, dagg], dt.bfloat16,
                                            tag="gb")
                            nc.vector.tensor_copy(gb[:], gt[:])
                            gt = gb
                        gat_cache[g0] = gt
                    return gat_cache[g0], t - g0

                gat_cache = {}
                for (n0, ns, bs) in SUPERS:
                    agg_of = {}
                    if mode == 'pre':
                        aggT = atpool.tile([128, KI, ns], dt.float32, tag="aggT")
                    for b in bs:
                        tb0, tb1 = int(off[b]), int(off[b + 1])
                        ntile = tb1 - tb0
                        agg_sb = apool.tile([128, dagg], dt.float32, tag="aggsb")
                        for (h0, hw) in halves:
                            ap_ = aps.tile([128, hw], dt.float32, tag="aps")
                            for ti, t in enumerate(range(tb0, tb1)):
                                gt, tl = gather_tile(t)
                                nc.tensor.matmul(
                                    ap_[:],
                                    oneh_sb[:, t * 128:(t + 1) * 128],
                                    gt[:, tl, h0:h0 + hw],
                                    start=(ti == 0),
                                    stop=(ti == ntile - 1))
                            nc.vector.tensor_scalar(
                                agg_sb[:, h0:h0 + hw], ap_[:],
                                invc_sb[:, b:b + 1], None, ALU.mult)
                        if mode == 'pre':
                            boff = (b - bs[0]) * 128
                            for k in range(KI):
                                kw = min(128, dagg - k * 128)
                                tr128(aggT[:kw, k, boff:boff + 128],
                                      agg_sb[:, k * 128:k * 128 + kw], kw)
                        else:
                            agg_of[b] = agg_sb

                    for j in range(JD):
                        wr = stream_w(Wr_d[p], din, j)
                        if mode == 'pre':
                            wl = stream_w(Wl_d[p], dagg, j)
                        cp = cps.tile([128, ns], dt.float32, tag="cps")
                        for k in range(KI):
                            nc.tensor.matmul(
                                cp[:], wr[:kp, k, :], hT[:kp, k, n0:n0 + ns],
                                start=(k == 0), stop=False)
                        if mode == 'pre':
                            kia = max(dagg // 128, 1)
                            kpa = min(dagg, 128)
                            for k in range(kia):
                                nc.tensor.matmul(
                                    cp[:], wl[:kpa, k, :], aggT[:kpa, k, :ns],
                                    start=False, stop=(k == kia - 1))
                        else:
                            for bi, b in enumerate(bs):
                                nc.tensor.matmul(
                                    cp[:, bi * 128:(bi + 1) * 128],
                                    agg_of[b][:, j * 128:(j + 1) * 128],
                                    ident_sb[:], is_transpose=True,
                                    start=False, stop=(bi == len(bs) - 1))
                        # epilogue: bias + relu + stats, or plain store
                        bias_ap = aux_sb[f"b{p}"][:, j:j + 1] if has_b else 0.0
                        if has_bn:
                            if n0 < 1024:
                                sl = [(0, ns, n0 // 512)]
                            else:
                                sw0 = NREAL7 - 1024
                                sl = [(0, sw0, 2), (sw0, ns - sw0, 3)]
                            for (s0, sw, slot) in sl:
                                nc.scalar.activation(
                                    raw[:, j, n0 + s0:n0 + s0 + sw],
                                    cp[:, s0:s0 + sw], ACT.Relu,
                                    bias=bias_ap,
                                    accum_out=stats_acc[:, j, slot:slot + 1])
                                sq = sqpool.tile([128, 512], dt.float32, tag="sq")
                                nc.scalar.activation(
                                    sq[:, :sw],
                                    raw[:, j, n0 + s0:n0 + s0 + sw], ACT.Square,
                                    accum_out=stats_acc[:, j, 4 + slot:5 + slot])
                        else:
                            if has_b:
                                nc.vector.tensor_scalar(
                                    raw[:, j, n0:n0 + ns], cp[:], bias_ap, None,
                                    ALU.add)
                            else:
                                nc.vector.tensor_copy(raw[:, j, n0:n0 + ns], cp[:])

                # ---------- BN: stats allreduce + normalize ----------
                if has_bn:
                    stats_sb = stpool.tile([128, 2 * JD], dt.float32, tag="statsum")
                    for j in range(JD):
                        for half, base in ((0, 0), (JD, 4)):
                            t0 = spool.tile([128, 1], dt.float32, tag="sc")
                            nc.vector.tensor_scalar(
                                t0[:], stats_acc[:, j, base + 3:base + 4],
                                mask_sb[:, 0:1], None, ALU.mult)
                            nc.vector.tensor_tensor(
                                t0[:], t0[:], stats_acc[:, j, base:base + 1],
                                ALU.add)
                            nc.vector.tensor_tensor(
                                t0[:], t0[:], stats_acc[:, j, base + 1:base + 2],
                                ALU.add)
                            nc.vector.tensor_tensor(
                                stats_sb[:, half + j:half + j + 1], t0[:],
                                stats_acc[:, j, base + 2:base + 3], ALU.add)
                    arr = stpool.tile([128, 2 * JD], dt.float32, tag="statsum")
                    if os.environ.get("GNN_NO_AR"):
                        nc.vector.tensor_scalar(arr[:], stats_sb[:], float(NC),
                                                None, ALU.mult)
                    else:
                        # AllGather partials + local sum (cheaper than AllReduce)
                        nc.sync.dma_start(ar_in[p][:, :], stats_sb[:])
                        nc.gpsimd.collective_compute(
                            "AllGather", ALU.bypass,
                            ins=[ar_in[p][:, :]], outs=[ar_out[p][:, :, :]],
                            replica_groups=RG)
                        parts = stpool.tile([128, NC, 2 * JD], dt.float32,
                                            tag="statparts")
                        nc.sync.dma_start(
                            parts[:], ar_out[p].rearrange("r p s -> p r s")[:])
                        nc.vector.tensor_tensor(arr[:], parts[:, 0, :],
                                                parts[:, 1, :], ALU.add)
                        for r in range(2, NC):
                            nc.vector.tensor_tensor(arr[:], arr[:],
                                                    parts[:, r, :], ALU.add)
                    mu = spool.tile([128, JD], dt.float32, tag="mu")
                    nc.vector.tensor_scalar(mu[:], arr[:, 0:JD], 1.0 / N, None,
                                            ALU.mult)
                    va = spool.tile([128, JD], dt.float32, tag="va")
                    nc.vector.tensor_scalar(va[:], arr[:, JD:2 * JD], 1.0 / N,
                                            None, ALU.mult)
                    mu2 = spool.tile([128, JD], dt.float32, tag="mu2")
                    nc.scalar.square(mu2[:], mu[:])
                    nc.vector.tensor_tensor(va[:], va[:], mu2[:], ALU.subtract)
                    nc.vector.tensor_scalar(va[:], va[:], EPS, None, ALU.add)
                    nc.scalar.sqrt(va[:], va[:])
                    aa = spool.tile([128, JD], dt.float32, tag="aa")
                    nc.vector.reciprocal(aa[:], va[:])
                    if has_g:
                        nc.vector.tensor_tensor(aa[:], aa[:], aux_sb[f"g{p}"][:],
                                                ALU.mult)
                    cc = spool.tile([128, JD], dt.float32, tag="cc")
                    nc.vector.tensor_tensor(cc[:], mu[:], aa[:], ALU.mult)
                    nc.vector.tensor_scalar(cc[:], cc[:], -1.0, None, ALU.mult)
                    if has_beta:
                        nc.vector.tensor_tensor(cc[:], cc[:],
                                                aux_sb[f"beta{p}"][:], ALU.add)
                    for j in range(JD):
                        nc.vector.tensor_scalar(
                            raw[:, j, :], raw[:, j, :],
                            aa[:, j:j + 1], cc[:, j:j + 1], ALU.mult, ALU.add)
                hT = raw

            # ---------- final output: node-major [1280, 128] ----------
            for b in range(NBLK):
                trp = tps.tile([128, 128], dt.float32, tag="tps")
                nc.tensor.transpose(trp[:], hT[:, 0, b * 128:(b + 1) * 128],
                                    ident_sb[:])
                ot = spool.tile([128, 128], dt.float32, tag="ot")
                nc.vector.tensor_copy(ot[:], trp[:])
                nc.sync.dma_start(out_d[b * 128:(b + 1) * 128, :], ot[:])

    nc.compile()
    return nc


# ---------------------------------------------------------------- entry point
_CACHE = {}


def prep_all(x, edge_index, params):
    """Returns (build_args, in_maps) for the SPMD run."""
    x = np.asarray(x, np.float32)
    per_core, T_b, off, TT = _prep_graph(edge_index)

    P = {k: np.asarray(v, np.float32) for k, v in params.items()}
    has_b = any(np.any(P[f"b{li}"] != 0) for li in range(9))
    has_g = any(np.any(P[f"g{bi}"] != 1) for bi in range(7))
    has_beta = any(np.any(P[f"beta{bi}"] != 0) for bi in range(7))

    x_pad = np.zeros((NPAD, FEAT), np.float32)
    x_pad[:N] = x
    ident = np.eye(128, dtype=np.float32)

    def col_wrap(v):
        d = v.shape[0]
        return np.ascontiguousarray(v.reshape(d // 128, 128).T)

    common = {"x_full": x_pad, "ident": ident}
    for p, (layers, din, dagg, dout, mode, has_bn) in enumerate(PASSES):
        common[f"Wlp{p}"] = np.ascontiguousarray(
            np.concatenate([P[f"Wl{li}"] for li in layers], 1))
        common[f"Wrp{p}"] = np.ascontiguousarray(
            np.concatenate([P[f"Wr{li}"] for li in layers], 1))
        if has_b:
            common[f"bcol{p}"] = col_wrap(np.concatenate(
                [P[f"b{li}"] for li in layers]))
        if has_bn:
            li = layers[0]
            if has_g:
                common[f"gcol{p}"] = col_wrap(P[f"g{li}"])
            if has_beta:
                common[f"betacol{p}"] = col_wrap(P[f"beta{li}"])

    in_maps = []
    for c in range(NC):
        idx_w, oneh, invc = per_core[c]
        m = dict(common)
        m["xT"] = np.ascontiguousarray(x_pad[c * NB:(c + 1) * NB].T)
        m["idx"] = idx_w
        m["oneh"] = oneh
        m["invc"] = invc
        m["maskc"] = np.full((128, 1), 1.0 if c < NC - 1 else 0.0, np.float32)
        in_maps.append(m)

    return (TT, T_b, off, (has_b, has_g, has_beta)), in_maps


def kernel(x, edge_index, params):
    from concourse.bass_utils import run_bass_kernel_spmd

    (TT, T_b, off, flags), in_maps = prep_all(x, edge_index, params)
    key = (TT, tuple(int(t) for t in T_b), flags)
    if key not in _CACHE:
        _CACHE[key] = _build(TT, T_b, off, flags)
    nc = _CACHE[key]

    res = run_bass_kernel_spmd(nc, in_maps, list(range(NC)), trace=False)
    out = np.concatenate([res.results[c]["out"] for c in range(NC)], 0)[:N]
    return (np.ascontiguousarray(out[:, :64]),
            np.ascontiguousarray(out[:, 64:128]))


# revision 21
# speedup vs baseline: 1.0155x; 1.0029x over previous
"""Distributed GNN encoder (9x SAGEConv + BN) on 8 Trainium2 NeuronCores.

Scheme (validated in numpy sim, rel_l2 ~4e-3 vs reference):
- Nodes sharded: core c owns padded rows [1280c, 1280(c+1)) of 10240 (10000 real).
- Feature-major compute on device; per conv pass either:
    'pre'  (expanding): AllGather h'' (bf16, width din), gather + one-hot
           scatter-matmul aggregation per 128-node block, then agg@Wl + h@Wr.
    'post' (contracting): Y = h''@Wl locally (f32), AllGather Y (bf16),
           aggregate Y, transpose agg into conv PSUM + h@Wr.
- Aggregation: dma_gather rows from the AllGathered DRAM buffer (bf16) +
  one-hot scatter matmuls (bf16, exact 0/1) into PSUM, exact f32 inv-degree.
- BatchNorm: relu with accumulated stats (f32) -> small AllReduce -> fused
  mult-add normalize. Dense matmuls all f32.
Outputs (mu, logvar) assembled host-side from per-core [1280,128] blocks.
"""
import sys
sys.path.insert(0, "/opt/trn_rl_repo")
import numpy as np
import ml_dtypes

NC = 8
BLK = 128
NBLK = 10
NB = BLK * NBLK          # 1280
NPAD = NC * NB           # 10240
N = 10000
FEAT = 64
EPS = 1e-5
NREAL7 = N - 7 * NB      # 1040: real rows of core 7 (stat-mask split point)

DIMS = [(64, 256), (256, 512), (512, 1024), (1024, 1024), (1024, 512),
        (512, 256), (256, 128), (128, 64), (128, 64)]

# (layers, din, dagg, dout, mode, has_bn)
PASSES = [
    ([0], 64, 64, 256, 'pre', True),
    ([1], 256, 256, 512, 'pre', True),
    ([2], 512, 512, 1024, 'pre', True),
    ([3], 1024, 1024, 1024, 'post', True),
    ([4], 1024, 512, 512, 'post', True),
    ([5], 512, 256, 256, 'post', True),
    ([6], 256, 128, 128, 'post', True),
    ([7, 8], 128, 128, 128, 'post', False),
]

# node-column chunks (free dim of conv matmuls) with their 128-node blocks;
# the last is split at the core-7 real/pad boundary for stats masking
SUPERS = [(0, 512, range(0, 4)), (512, 512, range(4, 8)), (1024, 256, range(8, 10))]


# ---------------------------------------------------------------- host prep
def _wrap_idxs(idx):
    idx = np.asarray(idx, np.int16)
    w = idx.reshape(-1, 16).T.copy()            # [16, n/16]
    return np.tile(w, (8, 1)).astype(np.int16)  # [128, n/16]


def _prep_graph(edge_index):
    src = np.asarray(edge_index[0], np.int64)
    dst = np.asarray(edge_index[1], np.int64)
    cnt = np.bincount(dst, minlength=N).astype(np.float32)
    inv_full = (1.0 / np.maximum(cnt, 1.0)).astype(np.float32)

    core_of = dst // NB
    blk_of = (dst % NB) // BLK

    T_b = np.ones(NBLK, np.int64)
    for b in range(NBLK):
        for c in range(NC):
            e = int(((core_of == c) & (blk_of == b)).sum())
            T_b[b] = max(T_b[b], -(-e // BLK))
    TT = int(T_b.sum())
    off = np.concatenate([[0], np.cumsum(T_b)]).astype(np.int64)

    per_core = []
    for c in range(NC):
        idx_all = np.zeros(TT * BLK, np.int64)
        oh_all = np.zeros((BLK, TT * BLK), np.float32)
        for b in range(NBLK):
            sel = np.nonzero((core_of == c) & (blk_of == b))[0]
            sel = sel[np.argsort(dst[sel], kind='stable')]
            t0 = int(off[b])
            j = np.arange(len(sel))
            t = t0 + j // BLK
            r = j % BLK
            idx_all[t * BLK + r] = src[sel]
            np.add.at(oh_all, (r, t * BLK + dst[sel] - (c * NB + b * BLK)), 1.0)
        ic = np.zeros((BLK, NBLK), np.float32)
        for b in range(NBLK):
            gids = c * NB + b * BLK + np.arange(BLK)
            v = gids < N
            ic[v, b] = inv_full[gids[v]]
        per_core.append((_wrap_idxs(idx_all), oh_all.astype(ml_dtypes.bfloat16), ic))
    return per_core, T_b, off, TT


# ---------------------------------------------------------------- device build
def _build(TT, T_b, off, flags):
    import concourse.bacc as bacc
    import concourse.tile as tile
    import concourse.mybir as mybir
    dt = mybir.dt
    ALU = mybir.AluOpType
    ACT = mybir.ActivationFunctionType
    has_b, has_g, has_beta = flags

    nc = bacc.Bacc("TRN2", target_bir_lowering=False, debug=False,
                   num_devices=NC)

    # ---- external inputs ----
    x_full = nc.dram_tensor("x_full", [NPAD, FEAT], dt.float32, kind="ExternalInput")
    xT = nc.dram_tensor("xT", [FEAT, NB], dt.float32, kind="ExternalInput")
    idx_d = nc.dram_tensor("idx", [128, TT * 8], dt.int16, kind="ExternalInput")
    oneh_d = nc.dram_tensor("oneh", [128, TT * 128], dt.bfloat16, kind="ExternalInput")
    invc_d = nc.dram_tensor("invc", [128, NBLK], dt.float32, kind="ExternalInput")
    mask_d = nc.dram_tensor("maskc", [128, 1], dt.float32, kind="ExternalInput")
    ident_d = nc.dram_tensor("ident", [128, 128], dt.float32, kind="ExternalInput")
    Wl_d, Wr_d, aux_d = {}, {}, {}
    for p, (layers, din, dagg, dout, mode, has_bn) in enumerate(PASSES):
        Wl_d[p] = nc.dram_tensor(f"Wlp{p}", [din, dout], dt.float32, kind="ExternalInput")
        Wr_d[p] = nc.dram_tensor(f"Wrp{p}", [din, dout], dt.float32, kind="ExternalInput")
        jd = dout // 128
        if has_b:
            aux_d[f"b{p}"] = nc.dram_tensor(f"bcol{p}", [128, jd], dt.float32,
                                            kind="ExternalInput")
        if has_bn and has_g:
            aux_d[f"g{p}"] = nc.dram_tensor(f"gcol{p}", [128, jd], dt.float32,
                                            kind="ExternalInput")
        if has_bn and has_beta:
            aux_d[f"beta{p}"] = nc.dram_tensor(f"betacol{p}", [128, jd], dt.float32,
                                               kind="ExternalInput")
    out_d = nc.dram_tensor("out", [NB, 128], dt.float32, kind="ExternalOutput")

    # ---- dram internals (exchange + stat bounces) ----
    ag_in, ag_out, ar_in, ar_out = {}, {}, {}, {}
    for p, (layers, din, dagg, dout, mode, has_bn) in enumerate(PASSES):
        if p > 0:
            ag_in[p] = nc.dram_tensor(f"agin{p}", [NB, dagg], dt.bfloat16)
            ag_out[p] = nc.dram_tensor(f"agout{p}", [NPAD, dagg], dt.bfloat16,
                                       addr_space="Shared")
        if has_bn:
            jd = dout // 128
            ar_in[p] = nc.dram_tensor(f"arin{p}", [128, 2 * jd], dt.float32)
            ar_out[p] = nc.dram_tensor(f"arout{p}", [NC, 128, 2 * jd], dt.float32,
                                       addr_space="Shared")

    RG = [list(range(NC))]

    with tile.TileContext(nc) as tc:
        with (
            tc.tile_pool(name="const", bufs=1) as cpool,
            tc.tile_pool(name="h", bufs=2) as hpool,
            tc.tile_pool(name="g", bufs=2) as gpool,
            tc.tile_pool(name="w", bufs=3) as wpool,
            tc.tile_pool(name="aggsb", bufs=4) as apool,
            tc.tile_pool(name="aggT", bufs=1) as atpool,
            tc.tile_pool(name="yj", bufs=2) as ypool,
            tc.tile_pool(name="small", bufs=4) as spool,
            tc.tile_pool(name="stats", bufs=2) as stpool,
            tc.tile_pool(name="sq", bufs=2) as sqpool,
            tc.tile_pool(name="nm", bufs=NBLK + 2) as npool,
            tc.tile_pool(name="aps", bufs=3, space="PSUM") as aps,
            tc.tile_pool(name="cps", bufs=3, space="PSUM") as cps,
            tc.tile_pool(name="tps", bufs=2, space="PSUM") as tps,
        ):
            # constants
            idx_sb = cpool.tile([128, TT * 8], dt.int16)
            nc.sync.dma_start(idx_sb[:], idx_d[:])
            oneh_sb = cpool.tile([128, TT * 128], dt.bfloat16)
            nc.sync.dma_start(oneh_sb[:], oneh_d[:])
            invc_sb = cpool.tile([128, NBLK], dt.float32)
            nc.sync.dma_start(invc_sb[:], invc_d[:])
            mask_sb = cpool.tile([128, 1], dt.float32)
            nc.sync.dma_start(mask_sb[:], mask_d[:])
            ident_sb = cpool.tile([128, 128], dt.float32)
            nc.sync.dma_start(ident_sb[:], ident_d[:])
            aux_sb = {}
            for k, d in aux_d.items():
                t = cpool.tile(list(d.shape), dt.float32)
                nc.sync.dma_start(t[:], d[:])
                aux_sb[k] = t

            # initial h: x^T block (feature-major, 64 real partitions)
            hT = hpool.tile([128, 1, NB], dt.float32, tag="h")
            nc.sync.dma_start(hT[:FEAT, 0, :], xT[:])

            def stream_w(dram, din, j):
                """Load W[:, j*128:(j+1)*128] as a [128, KI, 128] f32 tile."""
                kp = min(din, 128)
                ki = max(din // 128, 1)
                wt = wpool.tile([128, ki, 128], dt.float32, tag="w")
                v = dram.rearrange("(kc p) d -> p kc d", p=kp)
                nc.sync.dma_start(wt[:kp, :, :], v[:, :, j * 128:(j + 1) * 128])
                return wt

            def tr128(dst_ap, src_ap, kw=128):
                """dst = src^T via PE transpose + copy (f32)."""
                trp = tps.tile([128, 128], dt.float32, tag="tps")
                nc.tensor.transpose(trp[:kw, :], src_ap, ident_sb[:])
                nc.vector.tensor_copy(dst_ap, trp[:kw, :])

            import os
            npass = int(os.environ.get("GNN_NPASS", len(PASSES)))
            for p, (layers, din, dagg, dout, mode, has_bn) in enumerate(
                    PASSES[:npass]):
                kp = min(din, 128)
                KI = max(din // 128, 1)
                JD = dout // 128

                # ---------- exchange payload ----------
                # per-block node-major staging tiles: ONE DMA per block
                # instead of one per (j, block) (SP-sequencer issue cost
                # dominated the profile)
                if p > 0:
                    hwn = min(dagg, 512)
                    nm = {}

                    def nm_write(b, j, trp):
                        """Stage transpose chunk; flush the 512-col half to
                        ag_in when complete (one DMA per block-half)."""
                        h = (j * 128) // 512
                        if (b, h) not in nm:
                            nm[(b, h)] = npool.tile([128, hwn], dt.bfloat16,
                                                    tag="nm", name=f"nm{p}_{b}_{h}")
                        c0 = j * 128 - h * 512
                        nc.vector.tensor_copy(nm[(b, h)][:, c0:c0 + 128], trp)
                        if c0 + 128 == hwn or (j + 1) * 128 == dagg:
                            nc.sync.dma_start(
                                ag_in[p][b * 128:(b + 1) * 128,
                                         h * 512:h * 512 + c0 + 128],
                                nm.pop((b, h))[:, :c0 + 128])
                if mode == 'post':
                    # Y^T per j-chunk -> transpose -> bf16 -> staging
                    for j in range(JD):
                        wl = stream_w(Wl_d[p], din, j)
                        yj = ypool.tile([128, NB], dt.float32, tag="yj")
                        for (n0, ns, _bs) in SUPERS:
                            yp = cps.tile([128, ns], dt.float32, tag="cps")
                            for k in range(KI):
                                nc.tensor.matmul(
                                    yp[:], wl[:kp, k, :], hT[:kp, k, n0:n0 + ns],
                                    start=(k == 0), stop=(k == KI - 1))
                            nc.vector.tensor_copy(yj[:, n0:n0 + ns], yp[:])
                        for b in range(NBLK):
                            trp = tps.tile([128, 128], dt.float32, tag="tps")
                            nc.tensor.transpose(
                                trp[:], yj[:, b * 128:(b + 1) * 128], ident_sb[:])
                            nm_write(b, j, trp[:])
                elif p > 0:
                    # payload is h'' itself (din wide)
                    for j in range(KI):
                        for b in range(NBLK):
                            trp = tps.tile([128, 128], dt.float32, tag="tps")
                            nc.tensor.transpose(
                                trp[:], hT[:, j, b * 128:(b + 1) * 128], ident_sb[:])
                            nm_write(b, j, trp[:])
                if p > 0:
                    nc.gpsimd.collective_compute(
                        "AllGather", ALU.bypass,
                        ins=[ag_in[p][:, :]], outs=[ag_out[p][:, :]],
                        replica_groups=RG)
                    gsrc, gdt = ag_out[p], dt.bfloat16
                else:
                    gsrc, gdt = x_full, dt.float32

                # ---------- per-super aggregation + conv ----------
                raw = hpool.tile([128, JD, NB], dt.float32, tag="h")
                stats_acc = stpool.tile([128, JD, 8], dt.float32, tag="stats")
                halves = [(h * 512, min(512, dagg - h * 512))
                          for h in range((dagg + 511) // 512)]

                # flat gather groups across block boundaries (dma_gather caps
                # at 1024 idxs/instruction on HW); tiles of one group may
                # belong to different aggregation blocks
                GRP = 8

                def gather_tile(t):
                    """Return (tile_handle, local_index) for global tile t,
                    gathering its group on first touch."""
                    g0 = (t // GRP) * GRP
                    if g0 not in gat_cache:
                        gn = min(GRP, TT - g0)
                        gt = gpool.tile([128, gn, dagg], gdt, tag="g")
                        if os.environ.get("GNN_NO_GATHER"):
                            nc.vector.memset(gt[:], 0.0)
                        else:
                            nc.gpsimd.dma_gather(
                                gt[:], gsrc[:, :],
                                idx_sb[:, g0 * 8:(g0 + gn) * 8],
                                gn * 128, gn * 128, dagg)
                        if gdt != dt.bfloat16:
                            gb = gpool.tile([128, gn, dagg], dt.bfloat16,
                                            tag="gb")
                            nc.vector.tensor_copy(gb[:], gt[:])
                            gt = gb
                        gat_cache[g0] = gt
                    return gat_cache[g0], t - g0

                gat_cache = {}
                for (n0, ns, bs) in SUPERS:
                    agg_of = {}
                    if mode == 'pre':
                        aggT = atpool.tile([128, KI, ns], dt.float32, tag="aggT")
                    for b in bs:
                        tb0, tb1 = int(off[b]), int(off[b + 1])
                        ntile = tb1 - tb0
                        agg_sb = apool.tile([128, dagg], dt.float32, tag="aggsb")
                        for (h0, hw) in halves:
                            ap_ = aps.tile([128, hw], dt.float32, tag="aps")
                            for ti, t in enumerate(range(tb0, tb1)):
                                gt, tl = gather_tile(t)
                                nc.tensor.matmul(
                                    ap_[:],
                                    oneh_sb[:, t * 128:(t + 1) * 128],
                                    gt[:, tl, h0:h0 + hw],
                                    start=(ti == 0),
                                    stop=(ti == ntile - 1))
                            nc.vector.tensor_scalar(
                                agg_sb[:, h0:h0 + hw], ap_[:],
                                invc_sb[:, b:b + 1], None, ALU.mult)
                        if mode == 'pre':
                            boff = (b - bs[0]) * 128
                            for k in range(KI):
                                kw = min(128, dagg - k * 128)
                                tr128(aggT[:kw, k, boff:boff + 128],
                                      agg_sb[:, k * 128:k * 128 + kw], kw)
                        else:
                            agg_of[b] = agg_sb

                    for j in range(JD):
                        wr = stream_w(Wr_d[p], din, j)
                        if mode == 'pre':
                            wl = stream_w(Wl_d[p], dagg, j)
                        cp = cps.tile([128, ns], dt.float32, tag="cps")
                        for k in range(KI):
                            nc.tensor.matmul(
                                cp[:], wr[:kp, k, :], hT[:kp, k, n0:n0 + ns],
                                start=(k == 0), stop=False)
                        if mode == 'pre':
                            kia = max(dagg // 128, 1)
                            kpa = min(dagg, 128)
                            for k in range(kia):
                                nc.tensor.matmul(
                                    cp[:], wl[:kpa, k, :], aggT[:kpa, k, :ns],
                                    start=False, stop=(k == kia - 1))
                        else:
                            for bi, b in enumerate(bs):
                                nc.tensor.matmul(
                                    cp[:, bi * 128:(bi + 1) * 128],
                                    agg_of[b][:, j * 128:(j + 1) * 128],
                                    ident_sb[:], is_transpose=True,
                                    start=False, stop=(bi == len(bs) - 1))
                        # epilogue: bias + relu + stats, or plain store
                        bias_ap = aux_sb[f"b{p}"][:, j:j + 1] if has_b else 0.0
                        if has_bn:
                            if n0 < 1024:
                                sl = [(0, ns, n0 // 512)]
                            else:
                                sw0 = NREAL7 - 1024
                                sl = [(0, sw0, 2), (sw0, ns - sw0, 3)]
                            for (s0, sw, slot) in sl:
                                nc.scalar.activation(
                                    raw[:, j, n0 + s0:n0 + s0 + sw],
                                    cp[:, s0:s0 + sw], ACT.Relu,
                                    bias=bias_ap,
                                    accum_out=stats_acc[:, j, slot:slot + 1])
                                sq = sqpool.tile([128, 512], dt.float32, tag="sq")
                                nc.scalar.activation(
                                    sq[:, :sw],
                                    raw[:, j, n0 + s0:n0 + s0 + sw], ACT.Square,
                                    accum_out=stats_acc[:, j, 4 + slot:5 + slot])
                        else:
                            if has_b:
                                nc.vector.tensor_scalar(
                                    raw[:, j, n0:n0 + ns], cp[:], bias_ap, None,
                                    ALU.add)
                            else:
                                nc.vector.tensor_copy(raw[:, j, n0:n0 + ns], cp[:])

                # ---------- BN: stats allreduce + normalize ----------
                if has_bn:
                    stats_sb = stpool.tile([128, 2 * JD], dt.float32, tag="statsum")
                    for j in range(JD):
                        for half, base in ((0, 0), (JD, 4)):
                            t0 = spool.tile([128, 1], dt.float32, tag="sc")
                            nc.vector.tensor_scalar(
                                t0[:], stats_acc[:, j, base + 3:base + 4],
                                mask_sb[:, 0:1], None, ALU.mult)
                            nc.vector.tensor_tensor(
                                t0[:], t0[:], stats_acc[:, j, base:base + 1],
                                ALU.add)
                            nc.vector.tensor_tensor(
                                t0[:], t0[:], stats_acc[:, j, base + 1:base + 2],
                                ALU.add)
                            nc.vector.tensor_tensor(
                                stats_sb[:, half + j:half + j + 1], t0[:],
                                stats_acc[:, j, base + 2:base + 3], ALU.add)
                    arr = stpool.tile([128, 2 * JD], dt.float32, tag="statsum")
                    if os.environ.get("GNN_NO_AR"):
                        nc.vector.tensor_scalar(arr[:], stats_sb[:], float(NC),
                                                None, ALU.mult)
                    else:
                        # AllGather partials + local sum (cheaper than AllReduce)
                        nc.sync.dma_start(ar_in[p][:, :], stats_sb[:])
                        nc.gpsimd.collective_compute(
                            "AllGather", ALU.bypass,
                            ins=[ar_in[p][:, :]], outs=[ar_out[p][:, :, :]],
                            replica_groups=RG)
                        parts = stpool.tile([128, NC, 2 * JD], dt.float32,
                                            tag="statparts")
                        nc.sync.dma_start(
                            parts[:], ar_out[p].rearrange("r p s -> p r s")[:])
                        nc.vector.tensor_tensor(arr[:], parts[:, 0, :],
                                                parts[:, 1, :], ALU.add)
                        for r in range(2, NC):
                            nc.vector.tensor_tensor(arr[:], arr[:],
                                                    parts[:, r, :], ALU.add)
                    mu = spool.tile([128, JD], dt.float32, tag="mu")
                    nc.vector.tensor_scalar(mu[:], arr[:, 0:JD], 1.0 / N, None,
                                            ALU.mult)
                    va = spool.tile([128, JD], dt.float32, tag="va")
                    nc.vector.tensor_scalar(va[:], arr[:, JD:2 * JD], 1.0 / N,
                                            None, ALU.mult)
                    mu2 = spool.tile([128, JD], dt.float32, tag="mu2")
                    nc.scalar.square(mu2[:], mu[:])
                    nc.vector.tensor_tensor(va[:], va[:], mu2[:], ALU.subtract)
                    nc.vector.tensor_scalar(va[:], va[:], EPS, None, ALU.add)
                    nc.scalar.sqrt(va[:], va[:])
                    aa = spool.tile([128, JD], dt.float32, tag="aa")
                    nc.vector.reciprocal(aa[:], va[:])
                    if has_g:
                        nc.vector.tensor_tensor(aa[:], aa[:], aux_sb[f"g{p}"][:],
                                                ALU.mult)
                    cc = spool.tile([128, JD], dt.float32, tag="cc")
                    nc.vector.tensor_tensor(cc[:], mu[:], aa[:], ALU.mult)
                    nc.vector.tensor_scalar(cc[:], cc[:], -1.0, None, ALU.mult)
                    if has_beta:
                        nc.vector.tensor_tensor(cc[:], cc[:],
                                                aux_sb[f"beta{p}"][:], ALU.add)
                    for j in range(JD):
                        nc.vector.tensor_scalar(
                            raw[:, j, :], raw[:, j, :],
                            aa[:, j:j + 1], cc[:, j:j + 1], ALU.mult, ALU.add)
                hT = raw

            # ---------- final output: node-major [1280, 128] ----------
            for b in range(NBLK):
                trp = tps.tile([128, 128], dt.float32, tag="tps")
                nc.tensor.transpose(trp[:], hT[:, 0, b * 128:(b + 1) * 128],
                                    ident_sb[:])
                ot = spool.tile([128, 128], dt.float32, tag="ot")
                nc.vector.tensor_copy(ot[:], trp[:])
                nc.sync.dma_start(out_d[b * 128:(b + 1) * 128, :], ot[:])

    nc.compile()
    return nc


# ---------------------------------------------------------------- entry point
_CACHE = {}


def prep_all(x, edge_index, params):
    """Returns (build_args, in_maps) for the SPMD run."""
    x = np.asarray(x, np.float32)
    per_core, T_b, off, TT = _prep_graph(edge_index)

    P = {k: np.asarray(v, np.float32) for k, v in params.items()}
    has_b = any(np.any(P[f"b{li}"] != 0) for li in range(9))
    has_g = any(np.any(P[f"g{bi}"] != 1) for bi in range(7))
    has_beta = any(np.any(P[f"beta{bi}"] != 0) for bi in range(7))

    x_pad = np.zeros((NPAD, FEAT), np.float32)
    x_pad[:N] = x
    ident = np.eye(128, dtype=np.float32)

    def col_wrap(v):
        d = v.shape[0]
        return np.ascontiguousarray(v.reshape(d // 128, 128).T)

    common = {"x_full": x_pad, "ident": ident}
    for p, (layers, din, dagg, dout, mode, has_bn) in enumerate(PASSES):
        common[f"Wlp{p}"] = np.ascontiguousarray(
            np.concatenate([P[f"Wl{li}"] for li in layers], 1))
        common[f"Wrp{p}"] = np.ascontiguousarray(
            np.concatenate([P[f"Wr{li}"] for li in layers], 1))
        if has_b:
            common[f"bcol{p}"] = col_wrap(np.concatenate(
                [P[f"b{li}"] for li in layers]))
        if has_bn:
            li = layers[0]
            if has_g:
                common[f"gcol{p}"] = col_wrap(P[f"g{li}"])
            if has_beta:
                common[f"betacol{p}"] = col_wrap(P[f"beta{li}"])

    in_maps = []
    for c in range(NC):
        idx_w, oneh, invc = per_core[c]
        m = dict(common)
        m["xT"] = np.ascontiguousarray(x_pad[c * NB:(c + 1) * NB].T)
        m["idx"] = idx_w
        m["oneh"] = oneh
        m["invc"] = invc
        m["maskc"] = np.full((128, 1), 1.0 if c < NC - 1 else 0.0, np.float32)
        in_maps.append(m)

    return (TT, T_b, off, (has_b, has_g, has_beta)), in_maps


def kernel(x, edge_index, params):
    from concourse.bass_utils import run_bass_kernel_spmd

    (TT, T_b, off, flags), in_maps = prep_all(x, edge_index, params)
    key = (TT, tuple(int(t) for t in T_b), flags)
    if key not in _CACHE:
        _CACHE[key] = _build(TT, T_b, off, flags)
    nc = _CACHE[key]

    res = run_bass_kernel_spmd(nc, in_maps, list(range(NC)), trace=False)
    out = np.concatenate([res.results[c]["out"] for c in range(NC)], 0)[:N]
    return (np.ascontiguousarray(out[:, :64]),
            np.ascontiguousarray(out[:, 64:128]))
